# revision 14
# baseline (speedup 1.0000x reference)
"""Trainium2 Bass kernel for nn_AttMat_msg_lstm (GNN message passing + LSTM readout).

Sharding: data-parallel over batch dim B=8 -> 1 batch element per NeuronCore.
Per core: 3 GNN rounds over S=32 graphs of N=16 nodes (D=512), then an LSTM
over the S dimension with the 16 nodes as batch rows, then a linear readout.

Layout: feature dim D (=512) on partitions as 4 chunks of 128; (s, i, w)
flattened on the free dimension.  All matmuls run as float32r (full-rate
fp32 PE mode).  me = Wm_e @ edge is precomputed once to DRAM and streamed
back each round.  S is processed in two halves so the message tensor R
([128, 4, 4096] fp32) stays SBUF-resident across rounds.
"""

import sys

sys.path.insert(0, "/opt/trn_rl_repo")

import numpy as np
import ml_dtypes

import concourse.bass as bass
import concourse.bacc as bacc
import concourse.mybir as mybir
import concourse.tile as tile
from concourse.bass_utils import run_bass_kernel_spmd

F32 = mybir.dt.float32
F32R = mybir.dt.float32r
BF16 = mybir.dt.bfloat16
AF = mybir.ActivationFunctionType
AX = mybir.AxisListType
OP = mybir.AluOpType

B, S, N, D = 8, 32, 16, 512
HL, H, C, P = 512, 512, 6, 3
DC = D // 128          # 4 partition chunks of the feature dim
SN = S * N             # 512
SNN = S * N * N        # 8192
NHALF = 2
HSNN = SNN // NHALF    # 4096 free columns per half
HSN = SN // NHALF      # 256
BLKW = 512             # free-dim block (one PSUM bank of fp32)
NBLK = HSNN // BLKW    # 8 blocks per half
SPB = BLKW // (N * N)  # sequences per block (2)
NEG = -1.0e9


def r32(ap):
    return ap.bitcast(F32R)


def _ap(base, free_dims):
    """Rebuild an AP keeping base's partition dim, with explicit free dims."""
    return bass.AP(tensor=base.tensor, offset=base.offset,
                   ap=[list(base.ap[0])] + [list(d) for d in free_dims])


def load_w(nc, pool, ap_dram, kdim, mdim, dt=F32, name=None):
    t = pool.tile([128, kdim // 128, mdim], dt, tag=name)
    for k in range(kdim // 128):
        nc.sync.dma_start(out=t[:, k, :], in_=ap_dram[k * 128:(k + 1) * 128, :])
    return t


def build_kernel():
    nc = bacc.Bacc("TRN2", target_bir_lowering=False, debug=False)

    def din(name, shape, dt=F32):
        return nc.dram_tensor(name, shape, dt, kind="ExternalInput").ap()

    edge = din("edge", [D, SNN], F32R)            # premasked, [d, s*256+i*16+w]
    node0 = din("node0", [D, SN], F32R)           # premasked, [d, s*16+w]
    negmb2 = din("negmb2", [1, SNN], F32R)        # b2 + (1-emask)*NEG
    w1t = din("w1t", [D, HL], F32R)               # W1.T
    w2rep = din("w2rep", [HL, 128], F32R)         # W2 row replicated to 128 cols
    wmet = din("wmet", [D, D], F32R)              # Wm[:, D:].T
    wmht = din("wmht", [D, D], F32R)              # Wm[:, :D].T
    wiht = din("wiht", [D, 3 * D], BF16)          # Wih.T (bf16)
    whht = din("whht", [D, 3 * D], BF16)          # Whh.T (bf16)
    # consts: [:,0:512] nmask replicated; 512:516 b1; 516:520 bm;
    #         520:528 bih+bhh (rz); 528:532 bih(n); 532:536 bhh(n)
    consts = din("consts", [128, 536])
    ones1 = din("ones1", [1, 128], F32R)
    wliht = din("wliht", [D, 4 * H], F32R)        # Wl_ih.T
    wlhht = din("wlhht", [H, 4 * H], BF16)       # Wl_hh.T (bf16)
    blrow = din("blrow", [1, 4 * H], F32R)        # bl_ih + bl_hh
    wrt = din("wrt", [H, C], F32R)                # Wr.T
    brc = din("brc", [C, 1])
    idn16 = din("idn16", [16, 16])

    me_dram = nc.dram_tensor("me_buf", [D, SNN], F32, kind="Internal").ap()
    gih_dram = nc.dram_tensor("gih_buf", [S, 16, 4 * H], F32, kind="Internal").ap()
    adj_out = nc.dram_tensor("adj_out", [1, SNN], F32, kind="ExternalOutput").ap()
    lab_out = nc.dram_tensor("lab_out", [C, SN], F32, kind="ExternalOutput").ap()

    with tile.TileContext(nc) as tc:
        import contextlib
        with contextlib.ExitStack() as ctx:
            persist = ctx.enter_context(tc.tile_pool(name="persist", bufs=1))

            w1t_sb = load_w(nc, persist, w1t, D, HL, F32R, name="w1t")
            wmht_sb = load_w(nc, persist, wmht, D, D, F32R, name="wmht")
            w2rep_sb = load_w(nc, persist, w2rep, HL, 128, F32R, name="w2rep")

            cst = persist.tile([128, 536], F32, tag="consts")
            nc.sync.dma_start(out=cst, in_=consts)
            nmask_rep = cst[:, 0:512]
            b1c_sb, bmc_sb = cst[:, 512:516], cst[:, 516:520]
            brz_sb, bin_sb, bhn_sb = cst[:, 520:528], cst[:, 528:532], cst[:, 532:536]
            ones1_sb = persist.tile([1, 128], F32R, tag="ones1")
            nc.sync.dma_start(out=ones1_sb, in_=ones1)
            idn_sb = persist.tile([16, 16], F32, tag="idn16")
            nc.sync.dma_start(out=idn_sb, in_=idn16)

            # node_state [128, dc, s*16+w]
            ns = persist.tile([128, DC, SN], F32R, tag="ns")
            for k in range(DC):
                nc.sync.dma_start(out=ns[:, k, :], in_=node0[k * 128:(k + 1) * 128, :])

            # ---------------- GNN ----------------
            with tc.tile_pool(name="gnn", bufs=1) as gnn, \
                 tc.tile_pool(name="ground", bufs=1) as ground, \
                 tc.tile_pool(name="gblk", bufs=2) as gblk, \
                 tc.tile_pool(name="gwp", bufs=3) as gwp, \
                 tc.tile_pool(name="gtmp", bufs=2) as gtmp, \
                 tc.tile_pool(name="hps", bufs=3, space="PSUM") as hps, \
                 tc.tile_pool(name="aps", bufs=2, space="PSUM") as aps, \
                 tc.tile_pool(name="mps", bufs=3, space="PSUM") as mps:

                # phase 0: me = Wm_e @ edge  -> DRAM
                with tc.tile_pool(name="wmetp", bufs=1) as wmetp:
                    wmet_sb = load_w(nc, wmetp, wmet, D, D, F32R, name="wmet")
                    for blk in range(SNN // BLKW):
                        e_sb = gblk.tile([128, DC, BLKW], F32R, tag="h_sb")
                        for k in range(DC):
                            nc.sync.dma_start(
                                out=e_sb[:, k, :],
                                in_=edge[k * 128:(k + 1) * 128, blk * BLKW:(blk + 1) * BLKW])
                        me_sb = gblk.tile([128, DC, BLKW], F32, tag="me_o")
                        for mc in range(DC):
                            ps = hps.tile([128, BLKW], F32, tag="h_ps")
                            for k in range(DC):
                                nc.tensor.matmul(ps, r32(wmet_sb[:, k, mc * 128:(mc + 1) * 128]),
                                                 r32(e_sb[:, k, :]),
                                                 start=(k == 0), stop=(k == DC - 1))
                            nc.scalar.activation(me_sb[:, mc, :], ps, AF.Copy)
                            nc.sync.dma_start(
                                out=me_dram[mc * 128:(mc + 1) * 128, blk * BLKW:(blk + 1) * BLKW],
                                in_=me_sb[:, mc, :])

                for hf in range(NHALF):
                    c0 = hf * HSNN       # edge-col offset of this half
                    n0 = hf * HSN        # node-col offset of this half

                    # per-block R/gate tiles -> fine-grained WAR deps so the
                    # elementwise pass of block b overlaps matmuls of b+1
                    Rt = [gnn.tile([128, DC, BLKW], F32R, tag=f"R{b}", name=f"R{hf}_{b}")
                          for b in range(NBLK)]
                    gt = [gnn.tile([128, BLKW], F32, tag=f"g{b}", name=f"g{hf}_{b}")
                          for b in range(NBLK)]
                    for blk in range(NBLK):
                        for k in range(DC):
                            nc.sync.dma_start(
                                out=Rt[blk][:, k, :],
                                in_=edge[k * 128:(k + 1) * 128,
                                         c0 + blk * BLKW:c0 + (blk + 1) * BLKW])

                    for p in range(P):
                        # mh_p = Wm_h @ ns (this half)
                        mh = ground.tile([128, DC, HSN], F32, tag="mh")
                        for mc in range(DC):
                            ps = mps.tile([128, HSN], F32, tag="g256")
                            for k in range(DC):
                                nc.tensor.matmul(ps, r32(wmht_sb[:, k, mc * 128:(mc + 1) * 128]),
                                                 r32(ns[:, k, n0:n0 + HSN]),
                                                 start=(k == 0), stop=(k == DC - 1))
                            nc.scalar.activation(mh[:, mc, :], ps, AF.Copy)

                        msum = ground.tile([128, DC, HSN], F32R, tag="msum")
                        for blk in range(NBLK):
                            fb = blk * BLKW
                            R = Rt[blk]
                            gate = gt[blk]
                            # --- A: h = relu(W1 @ R + b1); adj; gate ---
                            h_sb = gblk.tile([128, DC, BLKW], F32R, tag="h_sb")
                            for mc in range(DC):
                                ps = hps.tile([128, BLKW], F32, tag="h_ps")
                                for k in range(DC):
                                    nc.tensor.matmul(
                                        ps, r32(w1t_sb[:, k, mc * 128:(mc + 1) * 128]),
                                        r32(R[:, k, :]),
                                        start=(k == 0), stop=(k == DC - 1))
                                if mc < 2:
                                    nc.scalar.activation(h_sb[:, mc, :], ps, AF.Relu,
                                                         bias=b1c_sb[:, mc:mc + 1])
                                else:
                                    nc.vector.tensor_scalar(
                                        h_sb[:, mc, :], ps, b1c_sb[:, mc:mc + 1], 0.0,
                                        op0=OP.add, op1=OP.max)
                            nm_sb = gtmp.tile([1, BLKW], F32R, tag="nm_sb")
                            nc.sync.dma_start(out=nm_sb, in_=negmb2[:, c0 + fb:c0 + fb + BLKW])
                            aps_t = aps.tile([128, BLKW], F32, tag="adj_ps")
                            for mc in range(DC):
                                nc.tensor.matmul(aps_t, r32(w2rep_sb[:, mc, :]),
                                                 r32(h_sb[:, mc, :]),
                                                 start=(mc == 0), stop=False)
                            nc.tensor.matmul(aps_t, r32(ones1_sb), r32(nm_sb),
                                             start=False, stop=True)
                            nc.scalar.activation(gate, aps_t, AF.Sigmoid)
                            if p == P - 1:
                                nc.sync.dma_start(out=adj_out[:, c0 + fb:c0 + fb + BLKW],
                                                  in_=gate[0:1, :])

                            # --- B: R <- gate * relu(me + mh + bm); msum ---
                            for k in range(DC):
                                me_in = gblk.tile([128, BLKW], F32, tag="me_i")
                                nc.sync.dma_start(
                                    out=me_in,
                                    in_=me_dram[k * 128:(k + 1) * 128, c0 + fb:c0 + fb + BLKW])
                                q = R[:, k, :]
                                for sq in range(SPB):
                                    mh_b = _ap(mh[:, k, (blk * SPB + sq) * N:
                                               (blk * SPB + sq + 1) * N],
                                               [[0, N], [1, N]])
                                    nc.vector.scalar_tensor_tensor(
                                        q[:, sq * N * N:(sq + 1) * N * N],
                                        me_in[:, sq * N * N:(sq + 1) * N * N],
                                        bmc_sb[:, k:k + 1], mh_b,
                                        op0=OP.add, op1=OP.add)
                                nc.scalar.activation(q, q, AF.Relu)
                                nc.gpsimd.tensor_mul(q, q, gate)
                                with nc.allow_low_precision(
                                        reason="float32r is fp32-width"):
                                    nc.vector.tensor_reduce(
                                        msum[:, k, blk * SPB * N:(blk + 1) * SPB * N],
                                        _ap(q, [[N, SPB * N], [1, N]]),
                                        axis=AX.X, op=OP.add)

                        # GRU update (weights streamed from DRAM per chunk)
                        msum_bf = ground.tile([128, DC, HSN], BF16, tag="msum_bf")
                        ns_bf = ground.tile([128, DC, HSN], BF16, tag="ns_bf")
                        for k in range(DC):
                            nc.scalar.activation(msum_bf[:, k, :], msum[:, k, :], AF.Copy)
                            nc.scalar.activation(ns_bf[:, k, :], ns[:, k, n0:n0 + HSN], AF.Copy)
                        r_sb = ground.tile([128, DC, HSN], F32, tag="r_sb")
                        z_sb = ground.tile([128, DC, HSN], F32, tag="z_sb")
                        n_sb = ground.tile([128, DC, HSN], F32, tag="n_sb")
                        for mc in range(8):
                            wi = gwp.tile([128, DC, 128], BF16, tag="wi")
                            wh = gwp.tile([128, DC, 128], BF16, tag="wh")
                            for k in range(DC):
                                nc.sync.dma_start(
                                    out=wi[:, k, :],
                                    in_=wiht[k * 128:(k + 1) * 128, mc * 128:(mc + 1) * 128])
                                nc.sync.dma_start(
                                    out=wh[:, k, :],
                                    in_=whht[k * 128:(k + 1) * 128, mc * 128:(mc + 1) * 128])
                            ps = mps.tile([128, HSN], F32, tag="g256")
                            for k in range(DC):
                                nc.tensor.matmul(ps, wi[:, k, :], msum_bf[:, k, :],
                                                 start=(k == 0), stop=False)
                            for k in range(DC):
                                nc.tensor.matmul(ps, wh[:, k, :],
                                                 ns_bf[:, k, :],
                                                 start=False, stop=(k == DC - 1))
                            dst = r_sb if mc < 4 else z_sb
                            nc.scalar.activation(dst[:, mc % 4, :], ps, AF.Sigmoid,
                                                 bias=brz_sb[:, mc:mc + 1])
                        for mc in range(DC):
                            wi = gwp.tile([128, DC, 128], BF16, tag="wi")
                            wh = gwp.tile([128, DC, 128], BF16, tag="wh")
                            for k in range(DC):
                                nc.sync.dma_start(
                                    out=wi[:, k, :],
                                    in_=wiht[k * 128:(k + 1) * 128, (8 + mc) * 128:(9 + mc) * 128])
                                nc.sync.dma_start(
                                    out=wh[:, k, :],
                                    in_=whht[k * 128:(k + 1) * 128, (8 + mc) * 128:(9 + mc) * 128])
                            ips = mps.tile([128, HSN], F32, tag="g256")
                            for k in range(DC):
                                nc.tensor.matmul(ips, wi[:, k, :], msum_bf[:, k, :],
                                                 start=(k == 0), stop=(k == DC - 1))
                            hps_t = mps.tile([128, HSN], F32, tag="g256")
                            for k in range(DC):
                                nc.tensor.matmul(hps_t, wh[:, k, :],
                                                 ns_bf[:, k, :],
                                                 start=(k == 0), stop=(k == DC - 1))
                            hn_sb = gtmp.tile([128, HSN], F32, tag="scratch")
                            nc.scalar.activation(hn_sb, hps_t, AF.Identity,
                                                 bias=bhn_sb[:, mc:mc + 1])
                            nc.vector.tensor_mul(hn_sb, r_sb[:, mc, :], hn_sb)
                            nc.vector.tensor_add(hn_sb, hn_sb, ips)
                            nc.scalar.activation(n_sb[:, mc, :], hn_sb, AF.Tanh,
                                                 bias=bin_sb[:, mc:mc + 1])
                        for k in range(DC):
                            u = gtmp.tile([128, HSN], F32, tag="scratch")
                            nc.vector.tensor_sub(u, ns[:, k, n0:n0 + HSN], n_sb[:, k, :])
                            nc.vector.tensor_mul(u, z_sb[:, k, :], u)
                            nc.vector.tensor_add(u, n_sb[:, k, :], u)
                            nc.vector.tensor_mul(ns[:, k, n0:n0 + HSN], u,
                                                 nmask_rep[:, n0:n0 + HSN])

            # ---------------- LSTM + readout ----------------
            with tc.tile_pool(name="lp", bufs=1) as lp, \
                 tc.tile_pool(name="ltmp", bufs=2) as lt, \
                 tc.tile_pool(name="lc", bufs=2) as lcp, \
                 tc.tile_pool(name="lpre", bufs=2) as lpre, \
                 tc.tile_pool(name="gps", bufs=6, space="PSUM") as gps, \
                 tc.tile_pool(name="tps", bufs=1, space="PSUM") as tps, \
                 tc.tile_pool(name="lps", bufs=1, space="PSUM") as lps:

                wliht_sb = load_w(nc, lp, wliht, D, 4 * H, F32R, name="wliht")
                wlhht_sb = load_w(nc, lp, wlhht, H, 4 * H, BF16, name="wlhht")
                bl_sb = lp.tile([1, 4 * H], F32R, tag="bl_sb")
                nc.sync.dma_start(out=bl_sb, in_=blrow)
                wrt_sb = load_w(nc, lp, wrt, H, C, F32R, name="wrt")
                brc_sb = lp.tile([C, 1], F32, tag="brc")
                nc.sync.dma_start(out=brc_sb, in_=brc)

                outsT = lp.tile([128, DC, SN], F32R, tag="outsT")
                # g_ih for all steps as one batch-128 GEMM, staged via DRAM
                # (step-major) so per-step reads start at partition 0.
                for g in range(4):
                    for fc in range(4):
                        ps = gps.tile([128, BLKW], F32, tag="g_ps")
                        nc.tensor.matmul(ps, r32(ones1_sb),
                                         r32(bl_sb[:, fc * BLKW:(fc + 1) * BLKW]),
                                         start=True, stop=False)
                        for k in range(DC):
                            nc.tensor.matmul(
                                ps, r32(ns[:, k, g * 128:(g + 1) * 128]),
                                r32(wliht_sb[:, k, fc * BLKW:(fc + 1) * BLKW]),
                                start=False, stop=(k == DC - 1))
                        gev = lt.tile([128, BLKW], F32, tag="gih_ev")
                        nc.vector.tensor_copy(gev, ps)
                        dst = bass.AP(tensor=gih_dram.tensor,
                                      offset=g * 8 * 16 * 4 * H + fc * BLKW,
                                      ap=[[16 * 4 * H, 8], [4 * H, 16], [1, BLKW]])
                        nc.sync.dma_start(out=dst, in_=gev)
                c_prev = lcp.tile([16, 4 * H], F32, tag="c")
                nc.vector.memset(c_prev, 0.0)

                for s in range(S):
                    g_pre = lpre.tile([16, 4 * H], F32, tag="g_pre")
                    nc.sync.dma_start(out=g_pre, in_=gih_dram[s])
                    g_sb = lt.tile([16, 4 * H], F32, tag="g_sb")
                    gih_s = g_pre
                    for fc in range(4):
                        gslc = slice(fc * BLKW, (fc + 1) * BLKW)
                        func = AF.Tanh if fc == 2 else AF.Sigmoid
                        if s == 0:
                            nc.scalar.activation(g_sb[:, gslc], gih_s[:, gslc], func)
                            continue
                        ps = gps.tile([16, BLKW], F32, tag="g_ps")
                        for k in range(DC):
                            nc.tensor.matmul(
                                ps, hT_prev[:, k, :],
                                wlhht_sb[:, k, fc * BLKW:(fc + 1) * BLKW],
                                start=(k == 0), stop=(k == DC - 1))
                        nc.vector.tensor_add(g_sb[:, gslc], ps, gih_s[:, gslc])
                        nc.scalar.activation(g_sb[:, gslc], g_sb[:, gslc], func)
                    i_g = g_sb[:, 0:H]
                    f_g = g_sb[:, H:2 * H]
                    g_g = g_sb[:, 2 * H:3 * H]
                    o_g = g_sb[:, 3 * H:4 * H]
                    c_new = lcp.tile([16, 4 * H], F32, tag="c")
                    nc.vector.tensor_mul(c_new[:, 0:H], f_g, c_prev[:, 0:H])
                    nc.vector.tensor_mul(c_new[:, H:2 * H], i_g, g_g)
                    nc.vector.tensor_add(c_new[:, 0:H], c_new[:, 0:H], c_new[:, H:2 * H])
                    h_sb = lt.tile([16, H], F32, tag="h_sb")
                    nc.scalar.activation(h_sb, c_new[:, 0:H], AF.Tanh)
                    nc.vector.tensor_mul(h_sb, o_g, h_sb)
                    c_prev = c_new
                    hT_bf = lpre.tile([128, DC, 16], BF16, tag="hT_bf")
                    for k in range(DC):
                        tp = tps.tile([128, 16], F32, tag="tp")
                        nc.tensor.transpose(tp, h_sb[:, k * 128:(k + 1) * 128], idn_sb)
                        nc.vector.tensor_copy(outsT[:, k, s * 16:(s + 1) * 16], tp)
                        nc.scalar.activation(hT_bf[:, k, :], tp, AF.Copy)
                    hT_prev = hT_bf

                lab_ps = lps.tile([C, SN], F32, tag="lab_ps")
                for k in range(DC):
                    nc.tensor.matmul(lab_ps, r32(wrt_sb[:, k, :]), r32(outsT[:, k, :]),
                                     start=(k == 0), stop=(k == DC - 1))
                lab_sb = lt.tile([C, SN], F32, tag="h_sb")
                nc.scalar.activation(lab_sb, lab_ps, AF.Identity, bias=brc_sb)
                nc.sync.dma_start(out=lab_out, in_=lab_sb)

    nc.compile()
    return nc


_NC = None


def get_nc():
    global _NC
    if _NC is None:
        _NC = build_kernel()
    return _NC


def prep_core_inputs(b, node_resnet, edge_resnet, node_num_rec, W1, b1, W2, b2,
                     Wm, bm, Wih, Whh, bih, bhh, Wl_ih, Wl_hh, bl_ih, bl_hh, Wr, br):
    f4 = np.float32
    nn_ = np.asarray(node_num_rec[b])                       # [S]
    mask = (np.arange(N)[None, :] < nn_[:, None])           # [S,N] bool
    emask = mask[:, :, None] & mask[:, None, :]             # [S,N,N]
    offdiag = ~np.eye(N, dtype=bool)

    node = np.asarray(node_resnet[b], f4) * mask[:, None, :]          # [S,D,N]
    edge = np.asarray(edge_resnet[b], f4) * (emask & offdiag)[:, None, :, :]

    edge_t = np.ascontiguousarray(edge.transpose(1, 0, 2, 3)).reshape(D, SNN)
    node_t = np.ascontiguousarray(node.transpose(1, 0, 2)).reshape(D, SN)
    emf = emask.astype(f4).reshape(1, SNN)
    negmb2 = (np.float32(b2[0]) + (1.0 - emf) * np.float32(NEG)).astype(f4)

    consts = np.zeros((128, 536), f4)
    consts[:, 0:512] = np.broadcast_to(mask.astype(f4).reshape(1, SN), (128, SN))
    consts[:, 512:516] = np.asarray(b1, f4).reshape(DC, 128).T
    consts[:, 516:520] = np.asarray(bm, f4).reshape(DC, 128).T
    consts[:, 520:528] = (np.asarray(bih, f4) + np.asarray(bhh, f4))[:1024].reshape(8, 128).T
    consts[:, 528:532] = np.asarray(bih, f4)[1024:].reshape(DC, 128).T
    consts[:, 532:536] = np.asarray(bhh, f4)[1024:].reshape(DC, 128).T

    ins = {
        "edge": edge_t, "node0": node_t, "negmb2": negmb2,
        "w1t": np.ascontiguousarray(np.asarray(W1, f4).T),
        "w2rep": np.ascontiguousarray(np.repeat(np.asarray(W2, f4).T, 128, axis=1)),
        "wmet": np.ascontiguousarray(np.asarray(Wm[:, D:], f4).T),
        "wmht": np.ascontiguousarray(np.asarray(Wm[:, :D], f4).T),
        "wiht": np.ascontiguousarray(np.asarray(Wih, f4).T).astype(ml_dtypes.bfloat16),
        "whht": np.ascontiguousarray(np.asarray(Whh, f4).T).astype(ml_dtypes.bfloat16),
        "consts": consts,
        "ones1": np.ones((1, 128), f4),
        "wliht": np.ascontiguousarray(np.asarray(Wl_ih, f4).T),
        "wlhht": np.ascontiguousarray(np.asarray(Wl_hh, f4).T).astype(ml_dtypes.bfloat16),
        "blrow": (np.asarray(bl_ih, f4) + np.asarray(bl_hh, f4)).reshape(1, 4 * H),
        "wrt": np.ascontiguousarray(np.asarray(Wr, f4).T),
        "brc": np.asarray(br, f4).reshape(C, 1),
        "idn16": np.eye(16, dtype=f4),
    }
    post = {"emask": emf.reshape(S, N, N), "nmask": mask}
    return ins, post


def kernel(node_resnet, edge_resnet, node_num_rec, W1, b1, W2, b2, Wm, bm,
           Wih, Whh, bih, bhh, Wl_ih, Wl_hh, bl_ih, bl_hh, Wr, br,
           _trace=False):
    nc = get_nc()
    args = (node_resnet, edge_resnet, node_num_rec, W1, b1, W2, b2, Wm, bm,
            Wih, Whh, bih, bhh, Wl_ih, Wl_hh, bl_ih, bl_hh, Wr, br)
    in_maps, posts = [], []
    for b in range(B):
        ins, post = prep_core_inputs(b, *args)
        in_maps.append(ins)
        posts.append(post)

    res = run_bass_kernel_spmd(nc, in_maps, core_ids=list(range(B)), trace=_trace)

    adj = np.zeros((B, S, N, N), np.float32)
    label = np.zeros((B, S, N, C), np.float32)
    for b in range(B):
        out = res.results[b]
        em = posts[b]["emask"]
        nm = posts[b]["nmask"]
        gate = out["adj_out"].reshape(S, N, N)
        adj[b] = gate + 0.5 * (1.0 - em)
        lab = out["lab_out"].reshape(C, S, N).transpose(1, 2, 0)
        label[b] = lab * nm[:, :, None]
    if _trace:
        kernel.last_exec_time_ns = res.exec_time_ns
        kernel.last_results = res
    return adj, label


# revision 15
# speedup vs baseline: 1.0163x; 1.0163x over previous
"""Trainium2 Bass kernel for nn_AttMat_msg_lstm (GNN message passing + LSTM readout).

Sharding: data-parallel over batch dim B=8 -> 1 batch element per NeuronCore.
Per core: 3 GNN rounds over S=32 graphs of N=16 nodes (D=512), then an LSTM
over the S dimension with the 16 nodes as batch rows, then a linear readout.

Layout: feature dim D (=512) on partitions as 4 chunks of 128; (s, i, w)
flattened on the free dimension.  All matmuls run as float32r (full-rate
fp32 PE mode).  me = Wm_e @ edge is precomputed once to DRAM and streamed
back each round.  S is processed in two halves so the message tensor R
([128, 4, 4096] fp32) stays SBUF-resident across rounds.
"""

import sys

sys.path.insert(0, "/opt/trn_rl_repo")

import numpy as np
import ml_dtypes

import concourse.bass as bass
import concourse.bacc as bacc
import concourse.mybir as mybir
import concourse.tile as tile
from concourse.bass_utils import run_bass_kernel_spmd

F32 = mybir.dt.float32
F32R = mybir.dt.float32r
BF16 = mybir.dt.bfloat16
AF = mybir.ActivationFunctionType
AX = mybir.AxisListType
OP = mybir.AluOpType

B, S, N, D = 8, 32, 16, 512
HL, H, C, P = 512, 512, 6, 3
DC = D // 128          # 4 partition chunks of the feature dim
SN = S * N             # 512
SNN = S * N * N        # 8192
NHALF = 2
HSNN = SNN // NHALF    # 4096 free columns per half
HSN = SN // NHALF      # 256
BLKW = 512             # free-dim block (one PSUM bank of fp32)
NBLK = HSNN // BLKW    # 8 blocks per half
SPB = BLKW // (N * N)  # sequences per block (2)
NEG = -1.0e9


def r32(ap):
    return ap.bitcast(F32R)


def _ap(base, free_dims):
    """Rebuild an AP keeping base's partition dim, with explicit free dims."""
    return bass.AP(tensor=base.tensor, offset=base.offset,
                   ap=[list(base.ap[0])] + [list(d) for d in free_dims])


def load_w(nc, pool, ap_dram, kdim, mdim, dt=F32, name=None):
    t = pool.tile([128, kdim // 128, mdim], dt, tag=name)
    for k in range(kdim // 128):
        nc.sync.dma_start(out=t[:, k, :], in_=ap_dram[k * 128:(k + 1) * 128, :])
    return t


def build_kernel():
    nc = bacc.Bacc("TRN2", target_bir_lowering=False, debug=False)

    def din(name, shape, dt=F32):
        return nc.dram_tensor(name, shape, dt, kind="ExternalInput").ap()

    edge = din("edge", [D, SNN], F32R)            # premasked, [d, s*256+i*16+w]
    node0 = din("node0", [D, SN], F32R)           # premasked, [d, s*16+w]
    negmb2 = din("negmb2", [1, SNN], F32R)        # b2 + (1-emask)*NEG
    w1t = din("w1t", [D, HL], F32R)               # W1.T
    w2rep = din("w2rep", [HL, 128], F32R)         # W2 row replicated to 128 cols
    wmet = din("wmet", [D, D], F32R)              # Wm[:, D:].T
    wmht = din("wmht", [D, D], F32R)              # Wm[:, :D].T
    wiht = din("wiht", [D, 3 * D], F32R)          # Wih.T
    whht = din("whht", [D, 3 * D], F32R)          # Whh.T
    # consts: [:,0:512] nmask replicated; 512:516 b1; 516:520 bm;
    #         520:528 bih+bhh (rz); 528:532 bih(n); 532:536 bhh(n)
    consts = din("consts", [128, 536])
    ones1 = din("ones1", [1, 128], F32R)
    wliht = din("wliht", [D, 4 * H], F32R)        # Wl_ih.T
    wlhht = din("wlhht", [H, 4 * H], F32R)        # Wl_hh.T
    blrow = din("blrow", [1, 4 * H], F32R)        # bl_ih + bl_hh
    wrt = din("wrt", [H, C], F32R)                # Wr.T
    brc = din("brc", [C, 1])
    idn16 = din("idn16", [16, 16])
    idn16r = din("idn16r", [16, 16], F32R)

    me_dram = nc.dram_tensor("me_buf", [D, SNN], F32, kind="Internal").ap()
    gih_dram = nc.dram_tensor("gih_buf", [S, 16, 4 * H], F32R, kind="Internal").ap()
    adj_out = nc.dram_tensor("adj_out", [1, SNN], F32, kind="ExternalOutput").ap()
    lab_out = nc.dram_tensor("lab_out", [C, SN], F32, kind="ExternalOutput").ap()

    with tile.TileContext(nc) as tc:
        import contextlib
        with contextlib.ExitStack() as ctx:
            persist = ctx.enter_context(tc.tile_pool(name="persist", bufs=1))

            w1t_sb = load_w(nc, persist, w1t, D, HL, F32R, name="w1t")
            wmht_sb = load_w(nc, persist, wmht, D, D, F32R, name="wmht")
            w2rep_sb = load_w(nc, persist, w2rep, HL, 128, F32R, name="w2rep")

            cst = persist.tile([128, 536], F32, tag="consts")
            nc.sync.dma_start(out=cst, in_=consts)
            nmask_rep = cst[:, 0:512]
            b1c_sb, bmc_sb = cst[:, 512:516], cst[:, 516:520]
            brz_sb, bin_sb, bhn_sb = cst[:, 520:528], cst[:, 528:532], cst[:, 532:536]
            ones1_sb = persist.tile([1, 128], F32R, tag="ones1")
            nc.sync.dma_start(out=ones1_sb, in_=ones1)
            idn_sb = persist.tile([16, 16], F32, tag="idn16")
            nc.sync.dma_start(out=idn_sb, in_=idn16)
            idnr_sb = persist.tile([16, 16], F32R, tag="idn16r")
            nc.sync.dma_start(out=idnr_sb, in_=idn16r)

            # node_state [128, dc, s*16+w]
            ns = persist.tile([128, DC, SN], F32R, tag="ns")
            for k in range(DC):
                nc.sync.dma_start(out=ns[:, k, :], in_=node0[k * 128:(k + 1) * 128, :])

            # ---------------- GNN ----------------
            with tc.tile_pool(name="gnn", bufs=1) as gnn, \
                 tc.tile_pool(name="ground", bufs=1) as ground, \
                 tc.tile_pool(name="gblk", bufs=2) as gblk, \
                 tc.tile_pool(name="gwp", bufs=3) as gwp, \
                 tc.tile_pool(name="gtmp", bufs=2) as gtmp, \
                 tc.tile_pool(name="hps", bufs=3, space="PSUM") as hps, \
                 tc.tile_pool(name="aps", bufs=2, space="PSUM") as aps, \
                 tc.tile_pool(name="mps", bufs=3, space="PSUM") as mps:

                # phase 0: me = Wm_e @ edge  -> DRAM
                with tc.tile_pool(name="wmetp", bufs=1) as wmetp:
                    wmet_sb = load_w(nc, wmetp, wmet, D, D, F32R, name="wmet")
                    for blk in range(SNN // BLKW):
                        e_sb = gblk.tile([128, DC, BLKW], F32R, tag="h_sb")
                        for k in range(DC):
                            nc.sync.dma_start(
                                out=e_sb[:, k, :],
                                in_=edge[k * 128:(k + 1) * 128, blk * BLKW:(blk + 1) * BLKW])
                        me_sb = gblk.tile([128, DC, BLKW], F32, tag="me_o")
                        for mc in range(DC):
                            ps = hps.tile([128, BLKW], F32, tag="h_ps")
                            for k in range(DC):
                                nc.tensor.matmul(ps, r32(wmet_sb[:, k, mc * 128:(mc + 1) * 128]),
                                                 r32(e_sb[:, k, :]),
                                                 start=(k == 0), stop=(k == DC - 1))
                            nc.scalar.activation(me_sb[:, mc, :], ps, AF.Copy)
                            nc.sync.dma_start(
                                out=me_dram[mc * 128:(mc + 1) * 128, blk * BLKW:(blk + 1) * BLKW],
                                in_=me_sb[:, mc, :])

                for hf in range(NHALF):
                    c0 = hf * HSNN       # edge-col offset of this half
                    n0 = hf * HSN        # node-col offset of this half

                    # per-block R/gate tiles -> fine-grained WAR deps so the
                    # elementwise pass of block b overlaps matmuls of b+1
                    Rt = [gnn.tile([128, DC, BLKW], F32R, tag=f"R{b}", name=f"R{hf}_{b}")
                          for b in range(NBLK)]
                    gt = [gnn.tile([128, BLKW], F32, tag=f"g{b}", name=f"g{hf}_{b}")
                          for b in range(NBLK)]
                    for blk in range(NBLK):
                        for k in range(DC):
                            nc.sync.dma_start(
                                out=Rt[blk][:, k, :],
                                in_=edge[k * 128:(k + 1) * 128,
                                         c0 + blk * BLKW:c0 + (blk + 1) * BLKW])

                    for p in range(P):
                        # mh_p = Wm_h @ ns (this half)
                        mh = ground.tile([128, DC, HSN], F32, tag="mh")
                        for mc in range(DC):
                            ps = mps.tile([128, HSN], F32, tag="g256")
                            for k in range(DC):
                                nc.tensor.matmul(ps, r32(wmht_sb[:, k, mc * 128:(mc + 1) * 128]),
                                                 r32(ns[:, k, n0:n0 + HSN]),
                                                 start=(k == 0), stop=(k == DC - 1))
                            nc.scalar.activation(mh[:, mc, :], ps, AF.Copy)

                        msum = ground.tile([128, DC, HSN], F32R, tag="msum")
                        for blk in range(NBLK):
                            fb = blk * BLKW
                            R = Rt[blk]
                            gate = gt[blk]
                            # --- A: h = relu(W1 @ R + b1); adj; gate ---
                            h_sb = gblk.tile([128, DC, BLKW], F32R, tag="h_sb")
                            for mc in range(DC):
                                ps = hps.tile([128, BLKW], F32, tag="h_ps")
                                for k in range(DC):
                                    nc.tensor.matmul(
                                        ps, r32(w1t_sb[:, k, mc * 128:(mc + 1) * 128]),
                                        r32(R[:, k, :]),
                                        start=(k == 0), stop=(k == DC - 1))
                                if mc < 2:
                                    nc.scalar.activation(h_sb[:, mc, :], ps, AF.Relu,
                                                         bias=b1c_sb[:, mc:mc + 1])
                                else:
                                    nc.vector.tensor_scalar(
                                        h_sb[:, mc, :], ps, b1c_sb[:, mc:mc + 1], 0.0,
                                        op0=OP.add, op1=OP.max)
                            nm_sb = gtmp.tile([1, BLKW], F32R, tag="nm_sb")
                            nc.sync.dma_start(out=nm_sb, in_=negmb2[:, c0 + fb:c0 + fb + BLKW])
                            aps_t = aps.tile([128, BLKW], F32, tag="adj_ps")
                            for mc in range(DC):
                                nc.tensor.matmul(aps_t, r32(w2rep_sb[:, mc, :]),
                                                 r32(h_sb[:, mc, :]),
                                                 start=(mc == 0), stop=False)
                            nc.tensor.matmul(aps_t, r32(ones1_sb), r32(nm_sb),
                                             start=False, stop=True)
                            nc.scalar.activation(gate, aps_t, AF.Sigmoid)
                            if p == P - 1:
                                nc.sync.dma_start(out=adj_out[:, c0 + fb:c0 + fb + BLKW],
                                                  in_=gate[0:1, :])

                            # --- B: R <- gate * relu(me + mh + bm); msum ---
                            for k in range(DC):
                                me_in = gblk.tile([128, BLKW], F32, tag="me_i")
                                nc.sync.dma_start(
                                    out=me_in,
                                    in_=me_dram[k * 128:(k + 1) * 128, c0 + fb:c0 + fb + BLKW])
                                q = R[:, k, :]
                                for sq in range(SPB):
                                    mh_b = _ap(mh[:, k, (blk * SPB + sq) * N:
                                               (blk * SPB + sq + 1) * N],
                                               [[0, N], [1, N]])
                                    nc.vector.scalar_tensor_tensor(
                                        q[:, sq * N * N:(sq + 1) * N * N],
                                        me_in[:, sq * N * N:(sq + 1) * N * N],
                                        bmc_sb[:, k:k + 1], mh_b,
                                        op0=OP.add, op1=OP.add)
                                nc.scalar.activation(q, q, AF.Relu)
                                nc.gpsimd.tensor_mul(q, q, gate)
                                with nc.allow_low_precision(
                                        reason="float32r is fp32-width"):
                                    nc.vector.tensor_reduce(
                                        msum[:, k, blk * SPB * N:(blk + 1) * SPB * N],
                                        _ap(q, [[N, SPB * N], [1, N]]),
                                        axis=AX.X, op=OP.add)

                        # GRU update (weights streamed from DRAM per chunk)
                        r_sb = ground.tile([128, DC, HSN], F32, tag="r_sb")
                        z_sb = ground.tile([128, DC, HSN], F32, tag="z_sb")
                        n_sb = ground.tile([128, DC, HSN], F32, tag="n_sb")
                        for mc in range(8):
                            wi = gwp.tile([128, DC, 128], F32R, tag="wi")
                            wh = gwp.tile([128, DC, 128], F32R, tag="wh")
                            for k in range(DC):
                                nc.sync.dma_start(
                                    out=wi[:, k, :],
                                    in_=wiht[k * 128:(k + 1) * 128, mc * 128:(mc + 1) * 128])
                                nc.sync.dma_start(
                                    out=wh[:, k, :],
                                    in_=whht[k * 128:(k + 1) * 128, mc * 128:(mc + 1) * 128])
                            ps = mps.tile([128, HSN], F32, tag="g256")
                            for k in range(DC):
                                nc.tensor.matmul(ps, r32(wi[:, k, :]), r32(msum[:, k, :]),
                                                 start=(k == 0), stop=False)
                            for k in range(DC):
                                nc.tensor.matmul(ps, r32(wh[:, k, :]),
                                                 r32(ns[:, k, n0:n0 + HSN]),
                                                 start=False, stop=(k == DC - 1))
                            dst = r_sb if mc < 4 else z_sb
                            nc.scalar.activation(dst[:, mc % 4, :], ps, AF.Sigmoid,
                                                 bias=brz_sb[:, mc:mc + 1])
                        for mc in range(DC):
                            wi = gwp.tile([128, DC, 128], F32R, tag="wi")
                            wh = gwp.tile([128, DC, 128], F32R, tag="wh")
                            for k in range(DC):
                                nc.sync.dma_start(
                                    out=wi[:, k, :],
                                    in_=wiht[k * 128:(k + 1) * 128, (8 + mc) * 128:(9 + mc) * 128])
                                nc.sync.dma_start(
                                    out=wh[:, k, :],
                                    in_=whht[k * 128:(k + 1) * 128, (8 + mc) * 128:(9 + mc) * 128])
                            ips = mps.tile([128, HSN], F32, tag="g256")
                            for k in range(DC):
                                nc.tensor.matmul(ips, r32(wi[:, k, :]), r32(msum[:, k, :]),
                                                 start=(k == 0), stop=(k == DC - 1))
                            hps_t = mps.tile([128, HSN], F32, tag="g256")
                            for k in range(DC):
                                nc.tensor.matmul(hps_t, r32(wh[:, k, :]),
                                                 r32(ns[:, k, n0:n0 + HSN]),
                                                 start=(k == 0), stop=(k == DC - 1))
                            hn_sb = gtmp.tile([128, HSN], F32, tag="scratch")
                            nc.scalar.activation(hn_sb, hps_t, AF.Identity,
                                                 bias=bhn_sb[:, mc:mc + 1])
                            nc.vector.tensor_mul(hn_sb, r_sb[:, mc, :], hn_sb)
                            nc.vector.tensor_add(hn_sb, hn_sb, ips)
                            nc.scalar.activation(n_sb[:, mc, :], hn_sb, AF.Tanh,
                                                 bias=bin_sb[:, mc:mc + 1])
                        for k in range(DC):
                            u = gtmp.tile([128, HSN], F32, tag="scratch")
                            nc.vector.tensor_sub(u, ns[:, k, n0:n0 + HSN], n_sb[:, k, :])
                            nc.vector.tensor_mul(u, z_sb[:, k, :], u)
                            nc.vector.tensor_add(u, n_sb[:, k, :], u)
                            nc.vector.tensor_mul(ns[:, k, n0:n0 + HSN], u,
                                                 nmask_rep[:, n0:n0 + HSN])

            # ---------------- LSTM + readout ----------------
            with tc.tile_pool(name="lp", bufs=1) as lp, \
                 tc.tile_pool(name="ltmp", bufs=2) as lt, \
                 tc.tile_pool(name="lc", bufs=2) as lcp, \
                 tc.tile_pool(name="lpre", bufs=2) as lpre, \
                 tc.tile_pool(name="gps", bufs=6, space="PSUM") as gps, \
                 tc.tile_pool(name="tps", bufs=1, space="PSUM") as tps, \
                 tc.tile_pool(name="lps", bufs=1, space="PSUM") as lps:

                wliht_sb = load_w(nc, lp, wliht, D, 4 * H, F32R, name="wliht")
                wlhht_sb = load_w(nc, lp, wlhht, H, 4 * H, F32R, name="wlhht")
                bl_sb = lp.tile([1, 4 * H], F32R, tag="bl_sb")
                nc.sync.dma_start(out=bl_sb, in_=blrow)
                wrt_sb = load_w(nc, lp, wrt, H, C, F32R, name="wrt")
                brc_sb = lp.tile([C, 1], F32, tag="brc")
                nc.sync.dma_start(out=brc_sb, in_=brc)

                outsT = lp.tile([128, DC, SN], F32R, tag="outsT")
                # g_ih for all steps as one batch-128 GEMM, staged via DRAM
                # (step-major) so per-step reads start at partition 0.
                for g in range(4):
                    for fc in range(4):
                        ps = gps.tile([128, BLKW], F32, tag="g_ps")
                        nc.tensor.matmul(ps, r32(ones1_sb),
                                         r32(bl_sb[:, fc * BLKW:(fc + 1) * BLKW]),
                                         start=True, stop=False)
                        for k in range(DC):
                            nc.tensor.matmul(
                                ps, r32(ns[:, k, g * 128:(g + 1) * 128]),
                                r32(wliht_sb[:, k, fc * BLKW:(fc + 1) * BLKW]),
                                start=False, stop=(k == DC - 1))
                        gev = lt.tile([128, BLKW], F32R, tag="gih_ev")
                        nc.vector.tensor_copy(gev, ps)
                        dst = bass.AP(tensor=gih_dram.tensor,
                                      offset=g * 8 * 16 * 4 * H + fc * BLKW,
                                      ap=[[16 * 4 * H, 8], [4 * H, 16], [1, BLKW]])
                        nc.sync.dma_start(out=dst, in_=gev)
                c_prev = lcp.tile([16, 4 * H], F32, tag="c")
                nc.vector.memset(c_prev, 0.0)

                for s in range(S):
                    g_pre = lpre.tile([16, 4 * H], F32R, tag="g_pre")
                    nc.sync.dma_start(out=g_pre, in_=gih_dram[s])
                    g_sb = lt.tile([16, 4 * H], F32, tag="g_sb")
                    gih_s = g_pre
                    for fc in range(4):
                        gslc = slice(fc * BLKW, (fc + 1) * BLKW)
                        func = AF.Tanh if fc == 2 else AF.Sigmoid
                        if s == 0:
                            nc.scalar.activation(g_sb[:, gslc], gih_s[:, gslc], func)
                            continue
                        ps = gps.tile([16, BLKW], F32, tag="g_ps")
                        nc.tensor.matmul(ps, idnr_sb, gih_s[:, gslc],
                                         start=True, stop=False)
                        for k in range(DC):
                            nc.tensor.matmul(
                                ps, r32(outsT[:, k, (s - 1) * 16:s * 16]),
                                r32(wlhht_sb[:, k, fc * BLKW:(fc + 1) * BLKW]),
                                start=False, stop=(k == DC - 1))
                        nc.scalar.activation(g_sb[:, gslc], ps, func)
                    i_g = g_sb[:, 0:H]
                    f_g = g_sb[:, H:2 * H]
                    g_g = g_sb[:, 2 * H:3 * H]
                    o_g = g_sb[:, 3 * H:4 * H]
                    c_new = lcp.tile([16, 4 * H], F32, tag="c")
                    nc.vector.tensor_mul(c_new[:, 0:H], f_g, c_prev[:, 0:H])
                    nc.vector.tensor_mul(c_new[:, H:2 * H], i_g, g_g)
                    nc.vector.tensor_add(c_new[:, 0:H], c_new[:, 0:H], c_new[:, H:2 * H])
                    h_sb = lt.tile([16, H], F32, tag="h_sb")
                    nc.scalar.activation(h_sb, c_new[:, 0:H], AF.Tanh)
                    nc.vector.tensor_mul(h_sb, o_g, h_sb)
                    c_prev = c_new
                    for k in range(DC):
                        tp = tps.tile([128, 16], F32, tag="tp")
                        nc.tensor.transpose(tp, h_sb[:, k * 128:(k + 1) * 128], idn_sb)
                        nc.vector.tensor_copy(outsT[:, k, s * 16:(s + 1) * 16], tp)

                lab_ps = lps.tile([C, SN], F32, tag="lab_ps")
                for k in range(DC):
                    nc.tensor.matmul(lab_ps, r32(wrt_sb[:, k, :]), r32(outsT[:, k, :]),
                                     start=(k == 0), stop=(k == DC - 1))
                lab_sb = lt.tile([C, SN], F32, tag="h_sb")
                nc.scalar.activation(lab_sb, lab_ps, AF.Identity, bias=brc_sb)
                nc.sync.dma_start(out=lab_out, in_=lab_sb)

    nc.compile()
    return nc


_NC = None


def get_nc():
    global _NC
    if _NC is None:
        _NC = build_kernel()
    return _NC


def prep_core_inputs(b, node_resnet, edge_resnet, node_num_rec, W1, b1, W2, b2,
                     Wm, bm, Wih, Whh, bih, bhh, Wl_ih, Wl_hh, bl_ih, bl_hh, Wr, br):
    f4 = np.float32
    nn_ = np.asarray(node_num_rec[b])                       # [S]
    mask = (np.arange(N)[None, :] < nn_[:, None])           # [S,N] bool
    emask = mask[:, :, None] & mask[:, None, :]             # [S,N,N]
    offdiag = ~np.eye(N, dtype=bool)

    node = np.asarray(node_resnet[b], f4) * mask[:, None, :]          # [S,D,N]
    edge = np.asarray(edge_resnet[b], f4) * (emask & offdiag)[:, None, :, :]

    edge_t = np.ascontiguousarray(edge.transpose(1, 0, 2, 3)).reshape(D, SNN)
    node_t = np.ascontiguousarray(node.transpose(1, 0, 2)).reshape(D, SN)
    emf = emask.astype(f4).reshape(1, SNN)
    negmb2 = (np.float32(b2[0]) + (1.0 - emf) * np.float32(NEG)).astype(f4)

    consts = np.zeros((128, 536), f4)
    consts[:, 0:512] = np.broadcast_to(mask.astype(f4).reshape(1, SN), (128, SN))
    consts[:, 512:516] = np.asarray(b1, f4).reshape(DC, 128).T
    consts[:, 516:520] = np.asarray(bm, f4).reshape(DC, 128).T
    consts[:, 520:528] = (np.asarray(bih, f4) + np.asarray(bhh, f4))[:1024].reshape(8, 128).T
    consts[:, 528:532] = np.asarray(bih, f4)[1024:].reshape(DC, 128).T
    consts[:, 532:536] = np.asarray(bhh, f4)[1024:].reshape(DC, 128).T

    ins = {
        "edge": edge_t, "node0": node_t, "negmb2": negmb2,
        "w1t": np.ascontiguousarray(np.asarray(W1, f4).T),
        "w2rep": np.ascontiguousarray(np.repeat(np.asarray(W2, f4).T, 128, axis=1)),
        "wmet": np.ascontiguousarray(np.asarray(Wm[:, D:], f4).T),
        "wmht": np.ascontiguousarray(np.asarray(Wm[:, :D], f4).T),
        "wiht": np.ascontiguousarray(np.asarray(Wih, f4).T),
        "whht": np.ascontiguousarray(np.asarray(Whh, f4).T),
        "consts": consts,
        "ones1": np.ones((1, 128), f4),
        "wliht": np.ascontiguousarray(np.asarray(Wl_ih, f4).T),
        "wlhht": np.ascontiguousarray(np.asarray(Wl_hh, f4).T),
        "blrow": (np.asarray(bl_ih, f4) + np.asarray(bl_hh, f4)).reshape(1, 4 * H),
        "wrt": np.ascontiguousarray(np.asarray(Wr, f4).T),
        "brc": np.asarray(br, f4).reshape(C, 1),
        "idn16": np.eye(16, dtype=f4),
        "idn16r": np.eye(16, dtype=f4),
    }
    post = {"emask": emf.reshape(S, N, N), "nmask": mask}
    return ins, post


def kernel(node_resnet, edge_resnet, node_num_rec, W1, b1, W2, b2, Wm, bm,
           Wih, Whh, bih, bhh, Wl_ih, Wl_hh, bl_ih, bl_hh, Wr, br,
           _trace=False):
    nc = get_nc()
    args = (node_resnet, edge_resnet, node_num_rec, W1, b1, W2, b2, Wm, bm,
            Wih, Whh, bih, bhh, Wl_ih, Wl_hh, bl_ih, bl_hh, Wr, br)
    in_maps, posts = [], []
    for b in range(B):
        ins, post = prep_core_inputs(b, *args)
        in_maps.append(ins)
        posts.append(post)

    res = run_bass_kernel_spmd(nc, in_maps, core_ids=list(range(B)), trace=_trace)

    adj = np.zeros((B, S, N, N), np.float32)
    label = np.zeros((B, S, N, C), np.float32)
    for b in range(B):
        out = res.results[b]
        em = posts[b]["emask"]
        nm = posts[b]["nmask"]
        gate = out["adj_out"].reshape(S, N, N)
        adj[b] = gate + 0.5 * (1.0 - em)
        lab = out["lab_out"].reshape(C, S, N).transpose(1, 2, 0)
        label[b] = lab * nm[:, :, None]
    if _trace:
        kernel.last_exec_time_ns = res.exec_time_ns
        kernel.last_results = res
    return adj, label


# revision 17
# speedup vs baseline: 1.0203x; 1.0040x over previous
"""Trainium2 Bass kernel for nn_AttMat_msg_lstm (GNN message passing + LSTM readout).

Sharding: data-parallel over batch dim B=8 -> 1 batch element per NeuronCore.
Per core: 3 GNN rounds over S=32 graphs of N=16 nodes (D=512), then an LSTM
over the S dimension with the 16 nodes as batch rows, then a linear readout.

Layout: feature dim D (=512) on partitions as 4 chunks of 128; (s, i, w)
flattened on the free dimension.  All matmuls run as float32r (full-rate
fp32 PE mode).  me = Wm_e @ edge is precomputed once to DRAM and streamed
back each round.  S is processed in two halves so the message tensor R
([128, 4, 4096] fp32) stays SBUF-resident across rounds.
"""

import sys

sys.path.insert(0, "/opt/trn_rl_repo")

import numpy as np
import ml_dtypes

import concourse.bass as bass
import concourse.bacc as bacc
import concourse.mybir as mybir
import concourse.tile as tile
from concourse.bass_utils import run_bass_kernel_spmd

F32 = mybir.dt.float32
F32R = mybir.dt.float32r
BF16 = mybir.dt.bfloat16
AF = mybir.ActivationFunctionType
AX = mybir.AxisListType
OP = mybir.AluOpType

B, S, N, D = 8, 32, 16, 512
HL, H, C, P = 512, 512, 6, 3
DC = D // 128          # 4 partition chunks of the feature dim
SN = S * N             # 512
SNN = S * N * N        # 8192
NHALF = 2
HSNN = SNN // NHALF    # 4096 free columns per half
HSN = SN // NHALF      # 256
BLKW = 512             # free-dim block (one PSUM bank of fp32)
NBLK = HSNN // BLKW    # 8 blocks per half
SPB = BLKW // (N * N)  # sequences per block (2)
NEG = -1.0e9


def r32(ap):
    return ap.bitcast(F32R)


def _ap(base, free_dims):
    """Rebuild an AP keeping base's partition dim, with explicit free dims."""
    return bass.AP(tensor=base.tensor, offset=base.offset,
                   ap=[list(base.ap[0])] + [list(d) for d in free_dims])


def load_w(nc, pool, ap_dram, kdim, mdim, dt=F32, name=None):
    t = pool.tile([128, kdim // 128, mdim], dt, tag=name)
    for k in range(kdim // 128):
        nc.sync.dma_start(out=t[:, k, :], in_=ap_dram[k * 128:(k + 1) * 128, :])
    return t


def build_kernel():
    nc = bacc.Bacc("TRN2", target_bir_lowering=False, debug=False)

    def din(name, shape, dt=F32):
        return nc.dram_tensor(name, shape, dt, kind="ExternalInput").ap()

    edge = din("edge", [D, SNN], F32R)            # premasked, [d, s*256+i*16+w]
    node0 = din("node0", [D, SN], F32R)           # premasked, [d, s*16+w]
    negmb2 = din("negmb2", [1, SNN], F32R)        # b2 + (1-emask)*NEG
    w1t = din("w1t", [D, HL], F32R)               # W1.T
    w2rep = din("w2rep", [HL, 128], F32R)         # W2 row replicated to 128 cols
    wmet = din("wmet", [D, D], F32R)              # Wm[:, D:].T
    wmht = din("wmht", [D, D], F32R)              # Wm[:, :D].T
    wiht = din("wiht", [D, 3 * D], F32R)          # Wih.T
    whht = din("whht", [D, 3 * D], F32R)          # Whh.T
    # consts: [:,0:512] nmask replicated; 512:516 b1; 516:520 bm;
    #         520:528 bih+bhh (rz); 528:532 bih(n); 532:536 bhh(n)
    consts = din("consts", [128, 536])
    ones1 = din("ones1", [1, 128], F32R)
    wliht = din("wliht", [D, 4 * H], F32R)        # Wl_ih.T
    wlhht = din("wlhht", [H, 4 * H], F32R)        # Wl_hh.T
    blrow = din("blrow", [1, 4 * H], F32R)        # bl_ih + bl_hh
    wrt = din("wrt", [H, C], F32R)                # Wr.T
    brc = din("brc", [C, 1])
    idn16 = din("idn16", [16, 16])
    idn16r = din("idn16r", [16, 16], F32R)

    me_dram = nc.dram_tensor("me_buf", [D, SNN], F32, kind="Internal").ap()
    gih_dram = nc.dram_tensor("gih_buf", [S, 16, 4 * H], F32R, kind="Internal").ap()
    adj_out = nc.dram_tensor("adj_out", [1, SNN], F32, kind="ExternalOutput").ap()
    lab_out = nc.dram_tensor("lab_out", [C, SN], F32, kind="ExternalOutput").ap()

    with tile.TileContext(nc) as tc:
        import contextlib
        with contextlib.ExitStack() as ctx:
            persist = ctx.enter_context(tc.tile_pool(name="persist", bufs=1))

            w1t_sb = load_w(nc, persist, w1t, D, HL, F32R, name="w1t")
            wmht_sb = load_w(nc, persist, wmht, D, D, F32R, name="wmht")
            w2rep_sb = load_w(nc, persist, w2rep, HL, 128, F32R, name="w2rep")

            cst = persist.tile([128, 536], F32, tag="consts")
            nc.sync.dma_start(out=cst, in_=consts)
            nmask_rep = cst[:, 0:512]
            b1c_sb, bmc_sb = cst[:, 512:516], cst[:, 516:520]
            brz_sb, bin_sb, bhn_sb = cst[:, 520:528], cst[:, 528:532], cst[:, 532:536]
            ones1_sb = persist.tile([1, 128], F32R, tag="ones1")
            nc.sync.dma_start(out=ones1_sb, in_=ones1)
            idn_sb = persist.tile([16, 16], F32, tag="idn16")
            nc.sync.dma_start(out=idn_sb, in_=idn16)
            idnr_sb = persist.tile([16, 16], F32R, tag="idn16r")
            nc.sync.dma_start(out=idnr_sb, in_=idn16r)

            # node_state [128, dc, s*16+w]
            ns = persist.tile([128, DC, SN], F32R, tag="ns")
            for k in range(DC):
                nc.sync.dma_start(out=ns[:, k, :], in_=node0[k * 128:(k + 1) * 128, :])

            # ---------------- GNN ----------------
            with tc.tile_pool(name="gnn", bufs=1) as gnn, \
                 tc.tile_pool(name="ground", bufs=1) as ground, \
                 tc.tile_pool(name="gblk", bufs=2) as gblk, \
                 tc.tile_pool(name="gwp", bufs=3) as gwp, \
                 tc.tile_pool(name="gtmp", bufs=2) as gtmp, \
                 tc.tile_pool(name="hps", bufs=4, space="PSUM") as hps, \
                 tc.tile_pool(name="aps", bufs=1, space="PSUM") as aps, \
                 tc.tile_pool(name="mps", bufs=3, space="PSUM") as mps:

                # phase 0: me = Wm_e @ edge  -> DRAM
                with tc.tile_pool(name="wmetp", bufs=1) as wmetp:
                    wmet_sb = load_w(nc, wmetp, wmet, D, D, F32R, name="wmet")
                    for blk in range(SNN // BLKW):
                        e_sb = gblk.tile([128, DC, BLKW], F32R, tag="h_sb")
                        for k in range(DC):
                            nc.sync.dma_start(
                                out=e_sb[:, k, :],
                                in_=edge[k * 128:(k + 1) * 128, blk * BLKW:(blk + 1) * BLKW])
                        me_sb = gblk.tile([128, DC, BLKW], F32, tag="me_o")
                        for mc in range(DC):
                            ps = hps.tile([128, BLKW], F32, tag="h_ps")
                            for k in range(DC):
                                nc.tensor.matmul(ps, r32(wmet_sb[:, k, mc * 128:(mc + 1) * 128]),
                                                 r32(e_sb[:, k, :]),
                                                 start=(k == 0), stop=(k == DC - 1))
                            nc.scalar.activation(me_sb[:, mc, :], ps, AF.Copy)
                            nc.sync.dma_start(
                                out=me_dram[mc * 128:(mc + 1) * 128, blk * BLKW:(blk + 1) * BLKW],
                                in_=me_sb[:, mc, :])

                for hf in range(NHALF):
                    c0 = hf * HSNN       # edge-col offset of this half
                    n0 = hf * HSN        # node-col offset of this half

                    # per-block R/gate tiles -> fine-grained WAR deps so the
                    # elementwise pass of block b overlaps matmuls of b+1
                    Rt = [gnn.tile([128, DC, BLKW], F32R, tag=f"R{b}", name=f"R{hf}_{b}")
                          for b in range(NBLK)]
                    gt = [gnn.tile([128, BLKW], F32, tag=f"g{b}", name=f"g{hf}_{b}")
                          for b in range(NBLK)]
                    for blk in range(NBLK):
                        for k in range(DC):
                            nc.sync.dma_start(
                                out=Rt[blk][:, k, :],
                                in_=edge[k * 128:(k + 1) * 128,
                                         c0 + blk * BLKW:c0 + (blk + 1) * BLKW])

                    for p in range(P):
                        # mh_p = Wm_h @ ns (this half)
                        mh = ground.tile([128, DC, HSN], F32, tag="mh")
                        for mc in range(DC):
                            ps = mps.tile([128, HSN], F32, tag="g256")
                            for k in range(DC):
                                nc.tensor.matmul(ps, r32(wmht_sb[:, k, mc * 128:(mc + 1) * 128]),
                                                 r32(ns[:, k, n0:n0 + HSN]),
                                                 start=(k == 0), stop=(k == DC - 1))
                            nc.scalar.activation(mh[:, mc, :], ps, AF.Copy)

                        msum = ground.tile([128, DC, HSN], F32R, tag="msum")
                        for blk in range(NBLK):
                            fb = blk * BLKW
                            R = Rt[blk]
                            gate = gt[blk]
                            # --- A: h = relu(W1 @ R + b1); adj; gate ---
                            h_sb = gblk.tile([128, DC, BLKW], F32R, tag="h_sb")
                            for mc in range(DC):
                                ps = hps.tile([128, BLKW], F32, tag="h_ps")
                                for k in range(DC):
                                    nc.tensor.matmul(
                                        ps, r32(w1t_sb[:, k, mc * 128:(mc + 1) * 128]),
                                        r32(R[:, k, :]),
                                        start=(k == 0), stop=(k == DC - 1))
                                if mc < 2:
                                    nc.scalar.activation(h_sb[:, mc, :], ps, AF.Relu,
                                                         bias=b1c_sb[:, mc:mc + 1])
                                else:
                                    nc.vector.tensor_scalar(
                                        h_sb[:, mc, :], ps, b1c_sb[:, mc:mc + 1], 0.0,
                                        op0=OP.add, op1=OP.max)
                            nm_sb = gtmp.tile([1, BLKW], F32R, tag="nm_sb")
                            nc.sync.dma_start(out=nm_sb, in_=negmb2[:, c0 + fb:c0 + fb + BLKW])
                            aps_t = aps.tile([128, BLKW], F32, tag="adj_ps")
                            for mc in range(DC):
                                nc.tensor.matmul(aps_t, r32(w2rep_sb[:, mc, :]),
                                                 r32(h_sb[:, mc, :]),
                                                 start=(mc == 0), stop=False)
                            nc.tensor.matmul(aps_t, r32(ones1_sb), r32(nm_sb),
                                             start=False, stop=True)
                            nc.scalar.activation(gate, aps_t, AF.Sigmoid)
                            if p == P - 1:
                                nc.sync.dma_start(out=adj_out[:, c0 + fb:c0 + fb + BLKW],
                                                  in_=gate[0:1, :])

                            # --- B: R <- gate * relu(me + mh + bm); msum ---
                            for k in range(DC):
                                me_in = gblk.tile([128, BLKW], F32, tag="me_i")
                                nc.sync.dma_start(
                                    out=me_in,
                                    in_=me_dram[k * 128:(k + 1) * 128, c0 + fb:c0 + fb + BLKW])
                                q = R[:, k, :]
                                for sq in range(SPB):
                                    mh_b = _ap(mh[:, k, (blk * SPB + sq) * N:
                                               (blk * SPB + sq + 1) * N],
                                               [[0, N], [1, N]])
                                    nc.vector.scalar_tensor_tensor(
                                        q[:, sq * N * N:(sq + 1) * N * N],
                                        me_in[:, sq * N * N:(sq + 1) * N * N],
                                        bmc_sb[:, k:k + 1], mh_b,
                                        op0=OP.add, op1=OP.add)
                                nc.scalar.activation(q, q, AF.Relu)
                                nc.gpsimd.tensor_mul(q, q, gate)
                                with nc.allow_low_precision(
                                        reason="float32r is fp32-width"):
                                    nc.vector.tensor_reduce(
                                        msum[:, k, blk * SPB * N:(blk + 1) * SPB * N],
                                        _ap(q, [[N, SPB * N], [1, N]]),
                                        axis=AX.X, op=OP.add)

                        # GRU update (weights streamed from DRAM per chunk)
                        r_sb = ground.tile([128, DC, HSN], F32, tag="r_sb")
                        z_sb = ground.tile([128, DC, HSN], F32, tag="z_sb")
                        n_sb = ground.tile([128, DC, HSN], F32, tag="n_sb")
                        for mc in range(8):
                            wi = gwp.tile([128, DC, 128], F32R, tag="wi")
                            wh = gwp.tile([128, DC, 128], F32R, tag="wh")
                            for k in range(DC):
                                nc.sync.dma_start(
                                    out=wi[:, k, :],
                                    in_=wiht[k * 128:(k + 1) * 128, mc * 128:(mc + 1) * 128])
                                nc.sync.dma_start(
                                    out=wh[:, k, :],
                                    in_=whht[k * 128:(k + 1) * 128, mc * 128:(mc + 1) * 128])
                            ps = mps.tile([128, HSN], F32, tag="g256")
                            for k in range(DC):
                                nc.tensor.matmul(ps, r32(wi[:, k, :]), r32(msum[:, k, :]),
                                                 start=(k == 0), stop=False)
                            for k in range(DC):
                                nc.tensor.matmul(ps, r32(wh[:, k, :]),
                                                 r32(ns[:, k, n0:n0 + HSN]),
                                                 start=False, stop=(k == DC - 1))
                            dst = r_sb if mc < 4 else z_sb
                            nc.scalar.activation(dst[:, mc % 4, :], ps, AF.Sigmoid,
                                                 bias=brz_sb[:, mc:mc + 1])
                        for mc in range(DC):
                            wi = gwp.tile([128, DC, 128], F32R, tag="wi")
                            wh = gwp.tile([128, DC, 128], F32R, tag="wh")
                            for k in range(DC):
                                nc.sync.dma_start(
                                    out=wi[:, k, :],
                                    in_=wiht[k * 128:(k + 1) * 128, (8 + mc) * 128:(9 + mc) * 128])
                                nc.sync.dma_start(
                                    out=wh[:, k, :],
                                    in_=whht[k * 128:(k + 1) * 128, (8 + mc) * 128:(9 + mc) * 128])
                            ips = mps.tile([128, HSN], F32, tag="g256")
                            for k in range(DC):
                                nc.tensor.matmul(ips, r32(wi[:, k, :]), r32(msum[:, k, :]),
                                                 start=(k == 0), stop=(k == DC - 1))
                            hps_t = mps.tile([128, HSN], F32, tag="g256")
                            for k in range(DC):
                                nc.tensor.matmul(hps_t, r32(wh[:, k, :]),
                                                 r32(ns[:, k, n0:n0 + HSN]),
                                                 start=(k == 0), stop=(k == DC - 1))
                            hn_sb = gtmp.tile([128, HSN], F32, tag="scratch")
                            nc.scalar.activation(hn_sb, hps_t, AF.Identity,
                                                 bias=bhn_sb[:, mc:mc + 1])
                            nc.vector.tensor_mul(hn_sb, r_sb[:, mc, :], hn_sb)
                            nc.vector.tensor_add(hn_sb, hn_sb, ips)
                            nc.scalar.activation(n_sb[:, mc, :], hn_sb, AF.Tanh,
                                                 bias=bin_sb[:, mc:mc + 1])
                        for k in range(DC):
                            u = gtmp.tile([128, HSN], F32, tag="scratch")
                            nc.vector.tensor_sub(u, ns[:, k, n0:n0 + HSN], n_sb[:, k, :])
                            nc.vector.tensor_mul(u, z_sb[:, k, :], u)
                            nc.vector.tensor_add(u, n_sb[:, k, :], u)
                            nc.vector.tensor_mul(ns[:, k, n0:n0 + HSN], u,
                                                 nmask_rep[:, n0:n0 + HSN])

            # ---------------- LSTM + readout ----------------
            with tc.tile_pool(name="lp", bufs=1) as lp, \
                 tc.tile_pool(name="ltmp", bufs=2) as lt, \
                 tc.tile_pool(name="lc", bufs=2) as lcp, \
                 tc.tile_pool(name="lpre", bufs=2) as lpre, \
                 tc.tile_pool(name="gps", bufs=6, space="PSUM") as gps, \
                 tc.tile_pool(name="tps", bufs=1, space="PSUM") as tps, \
                 tc.tile_pool(name="lps", bufs=1, space="PSUM") as lps:

                wliht_sb = load_w(nc, lp, wliht, D, 4 * H, F32R, name="wliht")
                wlhht_sb = load_w(nc, lp, wlhht, H, 4 * H, F32R, name="wlhht")
                bl_sb = lp.tile([1, 4 * H], F32R, tag="bl_sb")
                nc.sync.dma_start(out=bl_sb, in_=blrow)
                wrt_sb = load_w(nc, lp, wrt, H, C, F32R, name="wrt")
                brc_sb = lp.tile([C, 1], F32, tag="brc")
                nc.sync.dma_start(out=brc_sb, in_=brc)

                outsT = lp.tile([128, DC, SN], F32R, tag="outsT")
                # g_ih for all steps as one batch-128 GEMM, staged via DRAM
                # (step-major) so per-step reads start at partition 0.
                for g in range(4):
                    for fc in range(4):
                        ps = gps.tile([128, BLKW], F32, tag="g_ps")
                        nc.tensor.matmul(ps, r32(ones1_sb),
                                         r32(bl_sb[:, fc * BLKW:(fc + 1) * BLKW]),
                                         start=True, stop=False)
                        for k in range(DC):
                            nc.tensor.matmul(
                                ps, r32(ns[:, k, g * 128:(g + 1) * 128]),
                                r32(wliht_sb[:, k, fc * BLKW:(fc + 1) * BLKW]),
                                start=False, stop=(k == DC - 1))
                        gev = lt.tile([128, BLKW], F32R, tag="gih_ev")
                        nc.vector.tensor_copy(gev, ps)
                        dst = bass.AP(tensor=gih_dram.tensor,
                                      offset=g * 8 * 16 * 4 * H + fc * BLKW,
                                      ap=[[16 * 4 * H, 8], [4 * H, 16], [1, BLKW]])
                        nc.sync.dma_start(out=dst, in_=gev)
                c_prev = lcp.tile([16, 4 * H], F32, tag="c")
                nc.vector.memset(c_prev, 0.0)

                for s in range(S):
                    g_pre = lpre.tile([16, 4 * H], F32R, tag="g_pre")
                    nc.sync.dma_start(out=g_pre, in_=gih_dram[s])
                    g_sb = lt.tile([16, 4 * H], F32, tag="g_sb")
                    gih_s = g_pre
                    for fc in range(4):
                        gslc = slice(fc * BLKW, (fc + 1) * BLKW)
                        func = AF.Tanh if fc == 2 else AF.Sigmoid
                        if s == 0:
                            nc.scalar.activation(g_sb[:, gslc], gih_s[:, gslc], func)
                            continue
                        ps = gps.tile([16, BLKW], F32, tag="g_ps")
                        nc.tensor.matmul(ps, idnr_sb, gih_s[:, gslc],
                                         start=True, stop=False)
                        for k in range(DC):
                            nc.tensor.matmul(
                                ps, r32(outsT[:, k, (s - 1) * 16:s * 16]),
                                r32(wlhht_sb[:, k, fc * BLKW:(fc + 1) * BLKW]),
                                start=False, stop=(k == DC - 1))
                        nc.scalar.activation(g_sb[:, gslc], ps, func)
                    i_g = g_sb[:, 0:H]
                    f_g = g_sb[:, H:2 * H]
                    g_g = g_sb[:, 2 * H:3 * H]
                    o_g = g_sb[:, 3 * H:4 * H]
                    c_new = lcp.tile([16, 4 * H], F32, tag="c")
                    nc.vector.tensor_mul(c_new[:, 0:H], f_g, c_prev[:, 0:H])
                    nc.vector.tensor_mul(c_new[:, H:2 * H], i_g, g_g)
                    nc.vector.tensor_add(c_new[:, 0:H], c_new[:, 0:H], c_new[:, H:2 * H])
                    h_sb = lt.tile([16, H], F32, tag="h_sb")
                    nc.scalar.activation(h_sb, c_new[:, 0:H], AF.Tanh)
                    nc.vector.tensor_mul(h_sb, o_g, h_sb)
                    c_prev = c_new
                    for k in range(DC):
                        tp = tps.tile([128, 16], F32, tag="tp")
                        nc.tensor.transpose(tp, h_sb[:, k * 128:(k + 1) * 128], idn_sb)
                        nc.vector.tensor_copy(outsT[:, k, s * 16:(s + 1) * 16], tp)

                lab_ps = lps.tile([C, SN], F32, tag="lab_ps")
                for k in range(DC):
                    nc.tensor.matmul(lab_ps, r32(wrt_sb[:, k, :]), r32(outsT[:, k, :]),
                                     start=(k == 0), stop=(k == DC - 1))
                lab_sb = lt.tile([C, SN], F32, tag="h_sb")
                nc.scalar.activation(lab_sb, lab_ps, AF.Identity, bias=brc_sb)
                nc.sync.dma_start(out=lab_out, in_=lab_sb)

    nc.compile()
    return nc


_NC = None


def get_nc():
    global _NC
    if _NC is None:
        _NC = build_kernel()
    return _NC


def prep_core_inputs(b, node_resnet, edge_resnet, node_num_rec, W1, b1, W2, b2,
                     Wm, bm, Wih, Whh, bih, bhh, Wl_ih, Wl_hh, bl_ih, bl_hh, Wr, br):
    f4 = np.float32
    nn_ = np.asarray(node_num_rec[b])                       # [S]
    mask = (np.arange(N)[None, :] < nn_[:, None])           # [S,N] bool
    emask = mask[:, :, None] & mask[:, None, :]             # [S,N,N]
    offdiag = ~np.eye(N, dtype=bool)

    node = np.asarray(node_resnet[b], f4) * mask[:, None, :]          # [S,D,N]
    edge = np.asarray(edge_resnet[b], f4) * (emask & offdiag)[:, None, :, :]

    edge_t = np.ascontiguousarray(edge.transpose(1, 0, 2, 3)).reshape(D, SNN)
    node_t = np.ascontiguousarray(node.transpose(1, 0, 2)).reshape(D, SN)
    emf = emask.astype(f4).reshape(1, SNN)
    negmb2 = (np.float32(b2[0]) + (1.0 - emf) * np.float32(NEG)).astype(f4)

    consts = np.zeros((128, 536), f4)
    consts[:, 0:512] = np.broadcast_to(mask.astype(f4).reshape(1, SN), (128, SN))
    consts[:, 512:516] = np.asarray(b1, f4).reshape(DC, 128).T
    consts[:, 516:520] = np.asarray(bm, f4).reshape(DC, 128).T
    consts[:, 520:528] = (np.asarray(bih, f4) + np.asarray(bhh, f4))[:1024].reshape(8, 128).T
    consts[:, 528:532] = np.asarray(bih, f4)[1024:].reshape(DC, 128).T
    consts[:, 532:536] = np.asarray(bhh, f4)[1024:].reshape(DC, 128).T

    ins = {
        "edge": edge_t, "node0": node_t, "negmb2": negmb2,
        "w1t": np.ascontiguousarray(np.asarray(W1, f4).T),
        "w2rep": np.ascontiguousarray(np.repeat(np.asarray(W2, f4).T, 128, axis=1)),
        "wmet": np.ascontiguousarray(np.asarray(Wm[:, D:], f4).T),
        "wmht": np.ascontiguousarray(np.asarray(Wm[:, :D], f4).T),
        "wiht": np.ascontiguousarray(np.asarray(Wih, f4).T),
        "whht": np.ascontiguousarray(np.asarray(Whh, f4).T),
        "consts": consts,
        "ones1": np.ones((1, 128), f4),
        "wliht": np.ascontiguousarray(np.asarray(Wl_ih, f4).T),
        "wlhht": np.ascontiguousarray(np.asarray(Wl_hh, f4).T),
        "blrow": (np.asarray(bl_ih, f4) + np.asarray(bl_hh, f4)).reshape(1, 4 * H),
        "wrt": np.ascontiguousarray(np.asarray(Wr, f4).T),
        "brc": np.asarray(br, f4).reshape(C, 1),
        "idn16": np.eye(16, dtype=f4),
        "idn16r": np.eye(16, dtype=f4),
    }
    post = {"emask": emf.reshape(S, N, N), "nmask": mask}
    return ins, post


def kernel(node_resnet, edge_resnet, node_num_rec, W1, b1, W2, b2, Wm, bm,
           Wih, Whh, bih, bhh, Wl_ih, Wl_hh, bl_ih, bl_hh, Wr, br,
           _trace=False):
    nc = get_nc()
    args = (node_resnet, edge_resnet, node_num_rec, W1, b1, W2, b2, Wm, bm,
            Wih, Whh, bih, bhh, Wl_ih, Wl_hh, bl_ih, bl_hh, Wr, br)
    in_maps, posts = [], []
    for b in range(B):
        ins, post = prep_core_inputs(b, *args)
        in_maps.append(ins)
        posts.append(post)

    res = run_bass_kernel_spmd(nc, in_maps, core_ids=list(range(B)), trace=_trace)

    adj = np.zeros((B, S, N, N), np.float32)
    label = np.zeros((B, S, N, C), np.float32)
    for b in range(B):
        out = res.results[b]
        em = posts[b]["emask"]
        nm = posts[b]["nmask"]
        gate = out["adj_out"].reshape(S, N, N)
        adj[b] = gate + 0.5 * (1.0 - em)
        lab = out["lab_out"].reshape(C, S, N).transpose(1, 2, 0)
        label[b] = lab * nm[:, :, None]
    if _trace:
        kernel.last_exec_time_ns = res.exec_time_ns
        kernel.last_results = res
    return adj, label


# revision 18
# speedup vs baseline: 1.1806x; 1.1571x over previous
"""Trainium2 Bass kernel for nn_AttMat_msg_lstm (GNN message passing + LSTM readout).

Sharding: data-parallel over batch dim B=8 -> 1 batch element per NeuronCore.
Per core: 3 GNN rounds over S=32 graphs of N=16 nodes (D=512), then an LSTM
over the S dimension with the 16 nodes as batch rows, then a linear readout.

Layout: feature dim D (=512) on partitions as 4 chunks of 128; (s, i, w)
flattened on the free dimension.  All matmuls run as float32r (full-rate
fp32 PE mode).  me = Wm_e @ edge is precomputed once to DRAM and streamed
back each round.  S is processed in two halves so the message tensor R
([128, 4, 4096] fp32) stays SBUF-resident across rounds.
"""

import sys

sys.path.insert(0, "/opt/trn_rl_repo")

import numpy as np
import ml_dtypes

import concourse.bass as bass
import concourse.bacc as bacc
import concourse.mybir as mybir
import concourse.tile as tile
from concourse.bass_utils import run_bass_kernel_spmd

F32 = mybir.dt.float32
F32R = mybir.dt.float32r
BF16 = mybir.dt.bfloat16
AF = mybir.ActivationFunctionType
AX = mybir.AxisListType
OP = mybir.AluOpType

B, S, N, D = 8, 32, 16, 512
HL, H, C, P = 512, 512, 6, 3
DC = D // 128          # 4 partition chunks of the feature dim
SN = S * N             # 512
SNN = S * N * N        # 8192
NHALF = 2
HSNN = SNN // NHALF    # 4096 free columns per half
HSN = SN // NHALF      # 256
BLKW = 512             # free-dim block (one PSUM bank of fp32)
NBLK = HSNN // BLKW    # 8 blocks per half
SPB = BLKW // (N * N)  # sequences per block (2)
NEG = -1.0e9


def r32(ap):
    return ap.bitcast(F32R)


def _ap(base, free_dims):
    """Rebuild an AP keeping base's partition dim, with explicit free dims."""
    return bass.AP(tensor=base.tensor, offset=base.offset,
                   ap=[list(base.ap[0])] + [list(d) for d in free_dims])


def load_w(nc, pool, ap_dram, kdim, mdim, dt=F32, name=None):
    t = pool.tile([128, kdim // 128, mdim], dt, tag=name)
    for k in range(kdim // 128):
        nc.sync.dma_start(out=t[:, k, :], in_=ap_dram[k * 128:(k + 1) * 128, :])
    return t


def dma_wchunk(nc, dst, w_dram, kdim, col0, ncol):
    """One strided DMA: dst[p, k, c] = w_dram[k*128+p, col0+c]."""
    mrow = w_dram.shape[1]
    src_ap = bass.AP(tensor=w_dram.tensor, offset=col0,
                     ap=[[mrow, 128], [128 * mrow, kdim // 128], [1, ncol]])
    nc.sync.dma_start(out=dst, in_=src_ap)


def build_kernel():
    nc = bacc.Bacc("TRN2", target_bir_lowering=False, debug=False)

    def din(name, shape, dt=F32):
        return nc.dram_tensor(name, shape, dt, kind="ExternalInput").ap()

    edge = din("edge", [D, SNN], F32R)            # premasked, [d, s*256+i*16+w]
    node0 = din("node0", [D, SN], F32R)           # premasked, [d, s*16+w]
    negmb2 = din("negmb2", [1, SNN], F32R)        # b2 + (1-emask)*NEG
    w1t = din("w1t", [D, HL], F32R)               # W1.T
    w2rep = din("w2rep", [HL, 128], F32R)         # W2 row replicated to 128 cols
    wmet = din("wmet", [D, D], F32R)              # Wm[:, D:].T
    wmht = din("wmht", [D, D], F32R)              # Wm[:, :D].T
    wiht = din("wiht", [D, 3 * D], F32R)          # Wih.T
    whht = din("whht", [D, 3 * D], F32R)          # Whh.T
    # consts: [:,0:512] nmask replicated; 512:516 b1; 516:520 bm;
    #         520:528 bih+bhh (rz); 528:532 bih(n); 532:536 bhh(n)
    consts = din("consts", [128, 536])
    ones1 = din("ones1", [1, 128], F32R)
    wliht = din("wliht", [D, 4 * H], F32R)        # Wl_ih.T
    wlhht = din("wlhht", [H, 4 * H], F32R)        # Wl_hh.T
    blrow = din("blrow", [1, 4 * H], F32R)        # bl_ih + bl_hh
    wrt = din("wrt", [H, C], F32R)                # Wr.T
    brc = din("brc", [C, 1])
    idn16 = din("idn16", [16, 16])
    idn16r = din("idn16r", [16, 16], F32R)

    me_dram = nc.dram_tensor("me_buf", [D, SNN], F32, kind="Internal").ap()
    gih_dram = nc.dram_tensor("gih_buf", [S, 16, 4 * H], F32R, kind="Internal").ap()
    adj_out = nc.dram_tensor("adj_out", [1, SNN], F32, kind="ExternalOutput").ap()
    lab_out = nc.dram_tensor("lab_out", [C, SN], F32, kind="ExternalOutput").ap()

    with tile.TileContext(nc) as tc:
        import contextlib
        with contextlib.ExitStack() as ctx:
            persist = ctx.enter_context(tc.tile_pool(name="persist", bufs=1))

            w1t_sb = load_w(nc, persist, w1t, D, HL, F32R, name="w1t")
            wmht_sb = load_w(nc, persist, wmht, D, D, F32R, name="wmht")
            w2rep_sb = load_w(nc, persist, w2rep, HL, 128, F32R, name="w2rep")

            cst = persist.tile([128, 536], F32, tag="consts")
            nc.sync.dma_start(out=cst, in_=consts)
            nmask_rep = cst[:, 0:512]
            b1c_sb, bmc_sb = cst[:, 512:516], cst[:, 516:520]
            brz_sb, bin_sb, bhn_sb = cst[:, 520:528], cst[:, 528:532], cst[:, 532:536]
            ones1_sb = persist.tile([1, 128], F32R, tag="ones1")
            nc.sync.dma_start(out=ones1_sb, in_=ones1)
            idn_sb = persist.tile([16, 16], F32, tag="idn16")
            nc.sync.dma_start(out=idn_sb, in_=idn16)
            idnr_sb = persist.tile([16, 16], F32R, tag="idn16r")
            nc.sync.dma_start(out=idnr_sb, in_=idn16r)

            # node_state [128, dc, s*16+w]
            ns = persist.tile([128, DC, SN], F32R, tag="ns")
            for k in range(DC):
                nc.sync.dma_start(out=ns[:, k, :], in_=node0[k * 128:(k + 1) * 128, :])

            # ---------------- GNN ----------------
            with tc.tile_pool(name="gnn", bufs=1) as gnn, \
                 tc.tile_pool(name="ground", bufs=1) as ground, \
                 tc.tile_pool(name="gblk", bufs=2) as gblk, \
                 tc.tile_pool(name="gwp", bufs=3) as gwp, \
                 tc.tile_pool(name="gtmp", bufs=2) as gtmp, \
                 tc.tile_pool(name="hps", bufs=4, space="PSUM") as hps, \
                 tc.tile_pool(name="aps", bufs=1, space="PSUM") as aps, \
                 tc.tile_pool(name="mps", bufs=3, space="PSUM") as mps:

                # phase 0: me = Wm_e @ edge  -> DRAM
                with tc.tile_pool(name="wmetp", bufs=1) as wmetp:
                    wmet_sb = load_w(nc, wmetp, wmet, D, D, F32R, name="wmet")
                    for blk in range(SNN // BLKW):
                        e_sb = gblk.tile([128, DC, BLKW], F32R, tag="h_sb")
                        for k in range(DC):
                            nc.sync.dma_start(
                                out=e_sb[:, k, :],
                                in_=edge[k * 128:(k + 1) * 128, blk * BLKW:(blk + 1) * BLKW])
                        me_sb = gblk.tile([128, DC, BLKW], F32, tag="me_o")
                        for mc in range(DC):
                            ps = hps.tile([128, BLKW], F32, tag="h_ps")
                            for k in range(DC):
                                nc.tensor.matmul(ps, r32(wmet_sb[:, k, mc * 128:(mc + 1) * 128]),
                                                 r32(e_sb[:, k, :]),
                                                 start=(k == 0), stop=(k == DC - 1))
                            nc.scalar.activation(me_sb[:, mc, :], ps, AF.Copy)
                            nc.sync.dma_start(
                                out=me_dram[mc * 128:(mc + 1) * 128, blk * BLKW:(blk + 1) * BLKW],
                                in_=me_sb[:, mc, :])

                for hf in range(NHALF):
                    c0 = hf * HSNN       # edge-col offset of this half
                    n0 = hf * HSN        # node-col offset of this half

                    # per-block R/gate tiles -> fine-grained WAR deps so the
                    # elementwise pass of block b overlaps matmuls of b+1
                    Rt = [gnn.tile([128, DC, BLKW], F32R, tag=f"R{b}", name=f"R{hf}_{b}")
                          for b in range(NBLK)]
                    gt = [gnn.tile([128, BLKW], F32, tag=f"g{b}", name=f"g{hf}_{b}")
                          for b in range(NBLK)]
                    for blk in range(NBLK):
                        for k in range(DC):
                            nc.sync.dma_start(
                                out=Rt[blk][:, k, :],
                                in_=edge[k * 128:(k + 1) * 128,
                                         c0 + blk * BLKW:c0 + (blk + 1) * BLKW])

                    for p in range(P):
                        # mh_p = Wm_h @ ns (this half)
                        mh = ground.tile([128, DC, HSN], F32, tag="mh")
                        for mc in range(DC):
                            ps = mps.tile([128, HSN], F32, tag="g256")
                            for k in range(DC):
                                nc.tensor.matmul(ps, r32(wmht_sb[:, k, mc * 128:(mc + 1) * 128]),
                                                 r32(ns[:, k, n0:n0 + HSN]),
                                                 start=(k == 0), stop=(k == DC - 1))
                            nc.scalar.activation(mh[:, mc, :], ps, AF.Copy)

                        msum = ground.tile([128, DC, HSN], F32R, tag="msum")
                        for blk in range(NBLK):
                            fb = blk * BLKW
                            R = Rt[blk]
                            gate = gt[blk]
                            # --- A: h = relu(W1 @ R + b1); adj; gate ---
                            h_sb = gblk.tile([128, DC, BLKW], F32R, tag="h_sb")
                            for mc in range(DC):
                                ps = hps.tile([128, BLKW], F32, tag="h_ps")
                                for k in range(DC):
                                    nc.tensor.matmul(
                                        ps, r32(w1t_sb[:, k, mc * 128:(mc + 1) * 128]),
                                        r32(R[:, k, :]),
                                        start=(k == 0), stop=(k == DC - 1))
                                if mc < 2:
                                    nc.scalar.activation(h_sb[:, mc, :], ps, AF.Relu,
                                                         bias=b1c_sb[:, mc:mc + 1])
                                else:
                                    nc.vector.tensor_scalar(
                                        h_sb[:, mc, :], ps, b1c_sb[:, mc:mc + 1], 0.0,
                                        op0=OP.add, op1=OP.max)
                            nm_sb = gtmp.tile([1, BLKW], F32R, tag="nm_sb")
                            nc.sync.dma_start(out=nm_sb, in_=negmb2[:, c0 + fb:c0 + fb + BLKW])
                            aps_t = aps.tile([128, BLKW], F32, tag="adj_ps")
                            for mc in range(DC):
                                nc.tensor.matmul(aps_t, r32(w2rep_sb[:, mc, :]),
                                                 r32(h_sb[:, mc, :]),
                                                 start=(mc == 0), stop=False)
                            nc.tensor.matmul(aps_t, r32(ones1_sb), r32(nm_sb),
                                             start=False, stop=True)
                            nc.scalar.activation(gate, aps_t, AF.Sigmoid)
                            if p == P - 1:
                                nc.sync.dma_start(out=adj_out[:, c0 + fb:c0 + fb + BLKW],
                                                  in_=gate[0:1, :])

                            # --- B: R <- gate * relu(me + mh + bm); msum ---
                            for k in range(DC):
                                me_in = gblk.tile([128, BLKW], F32, tag="me_i")
                                nc.sync.dma_start(
                                    out=me_in,
                                    in_=me_dram[k * 128:(k + 1) * 128, c0 + fb:c0 + fb + BLKW])
                                q = R[:, k, :]
                                for sq in range(SPB):
                                    mh_b = _ap(mh[:, k, (blk * SPB + sq) * N:
                                               (blk * SPB + sq + 1) * N],
                                               [[0, N], [1, N]])
                                    nc.vector.scalar_tensor_tensor(
                                        q[:, sq * N * N:(sq + 1) * N * N],
                                        me_in[:, sq * N * N:(sq + 1) * N * N],
                                        bmc_sb[:, k:k + 1], mh_b,
                                        op0=OP.add, op1=OP.add)
                                nc.scalar.activation(q, q, AF.Relu)
                                nc.gpsimd.tensor_mul(q, q, gate)
                                with nc.allow_low_precision(
                                        reason="float32r is fp32-width"):
                                    nc.vector.tensor_reduce(
                                        msum[:, k, blk * SPB * N:(blk + 1) * SPB * N],
                                        _ap(q, [[N, SPB * N], [1, N]]),
                                        axis=AX.X, op=OP.add)

                        # GRU update (weights streamed from DRAM per chunk)
                        r_sb = ground.tile([128, DC, HSN], F32, tag="r_sb")
                        z_sb = ground.tile([128, DC, HSN], F32, tag="z_sb")
                        n_sb = ground.tile([128, DC, HSN], F32, tag="n_sb")
                        for mc in range(8):
                            wi = gwp.tile([128, DC, 128], F32R, tag="wi")
                            wh = gwp.tile([128, DC, 128], F32R, tag="wh")
                            dma_wchunk(nc, wi, wiht, D, mc * 128, 128)
                            dma_wchunk(nc, wh, whht, D, mc * 128, 128)
                            ps = mps.tile([128, HSN], F32, tag="g256")
                            for k in range(DC):
                                nc.tensor.matmul(ps, r32(wi[:, k, :]), r32(msum[:, k, :]),
                                                 start=(k == 0), stop=False)
                            for k in range(DC):
                                nc.tensor.matmul(ps, r32(wh[:, k, :]),
                                                 r32(ns[:, k, n0:n0 + HSN]),
                                                 start=False, stop=(k == DC - 1))
                            dst = r_sb if mc < 4 else z_sb
                            nc.scalar.activation(dst[:, mc % 4, :], ps, AF.Sigmoid,
                                                 bias=brz_sb[:, mc:mc + 1])
                        for mc in range(DC):
                            wi = gwp.tile([128, DC, 128], F32R, tag="wi")
                            wh = gwp.tile([128, DC, 128], F32R, tag="wh")
                            dma_wchunk(nc, wi, wiht, D, (8 + mc) * 128, 128)
                            dma_wchunk(nc, wh, whht, D, (8 + mc) * 128, 128)
                            ips = mps.tile([128, HSN], F32, tag="g256")
                            for k in range(DC):
                                nc.tensor.matmul(ips, r32(wi[:, k, :]), r32(msum[:, k, :]),
                                                 start=(k == 0), stop=(k == DC - 1))
                            hps_t = mps.tile([128, HSN], F32, tag="g256")
                            for k in range(DC):
                                nc.tensor.matmul(hps_t, r32(wh[:, k, :]),
                                                 r32(ns[:, k, n0:n0 + HSN]),
                                                 start=(k == 0), stop=(k == DC - 1))
                            hn_sb = gtmp.tile([128, HSN], F32, tag="scratch")
                            nc.scalar.activation(hn_sb, hps_t, AF.Identity,
                                                 bias=bhn_sb[:, mc:mc + 1])
                            nc.vector.tensor_mul(hn_sb, r_sb[:, mc, :], hn_sb)
                            nc.vector.tensor_add(hn_sb, hn_sb, ips)
                            nc.scalar.activation(n_sb[:, mc, :], hn_sb, AF.Tanh,
                                                 bias=bin_sb[:, mc:mc + 1])
                        for k in range(DC):
                            u = gtmp.tile([128, HSN], F32, tag="scratch")
                            nc.vector.tensor_sub(u, ns[:, k, n0:n0 + HSN], n_sb[:, k, :])
                            nc.vector.tensor_mul(u, z_sb[:, k, :], u)
                            nc.vector.tensor_add(u, n_sb[:, k, :], u)
                            nc.vector.tensor_mul(ns[:, k, n0:n0 + HSN], u,
                                                 nmask_rep[:, n0:n0 + HSN])

            # ---------------- LSTM + readout ----------------
            with tc.tile_pool(name="lp", bufs=1) as lp, \
                 tc.tile_pool(name="ltmp", bufs=2) as lt, \
                 tc.tile_pool(name="lc", bufs=2) as lcp, \
                 tc.tile_pool(name="lpre", bufs=2) as lpre, \
                 tc.tile_pool(name="gps", bufs=6, space="PSUM") as gps, \
                 tc.tile_pool(name="tps", bufs=1, space="PSUM") as tps, \
                 tc.tile_pool(name="lps", bufs=1, space="PSUM") as lps:

                wliht_sb = load_w(nc, lp, wliht, D, 4 * H, F32R, name="wliht")
                wlhht_sb = load_w(nc, lp, wlhht, H, 4 * H, F32R, name="wlhht")
                bl_sb = lp.tile([1, 4 * H], F32R, tag="bl_sb")
                nc.sync.dma_start(out=bl_sb, in_=blrow)
                wrt_sb = load_w(nc, lp, wrt, H, C, F32R, name="wrt")
                brc_sb = lp.tile([C, 1], F32, tag="brc")
                nc.sync.dma_start(out=brc_sb, in_=brc)

                outsT = lp.tile([128, DC, SN], F32R, tag="outsT")
                # g_ih for all steps as one batch-128 GEMM, staged via DRAM
                # (step-major) so per-step reads start at partition 0.
                for g in range(4):
                    for fc in range(4):
                        ps = gps.tile([128, BLKW], F32, tag="g_ps")
                        nc.tensor.matmul(ps, r32(ones1_sb),
                                         r32(bl_sb[:, fc * BLKW:(fc + 1) * BLKW]),
                                         start=True, stop=False)
                        for k in range(DC):
                            nc.tensor.matmul(
                                ps, r32(ns[:, k, g * 128:(g + 1) * 128]),
                                r32(wliht_sb[:, k, fc * BLKW:(fc + 1) * BLKW]),
                                start=False, stop=(k == DC - 1))
                        gev = lt.tile([128, BLKW], F32R, tag="gih_ev")
                        nc.vector.tensor_copy(gev, ps)
                        dst = bass.AP(tensor=gih_dram.tensor,
                                      offset=g * 8 * 16 * 4 * H + fc * BLKW,
                                      ap=[[16 * 4 * H, 8], [4 * H, 16], [1, BLKW]])
                        nc.sync.dma_start(out=dst, in_=gev)
                c_prev = lcp.tile([16, 4 * H], F32, tag="c")
                nc.vector.memset(c_prev, 0.0)

                for s in range(S):
                    g_pre = lpre.tile([16, 4 * H], F32R, tag="g_pre")
                    nc.sync.dma_start(out=g_pre, in_=gih_dram[s])
                    g_sb = lt.tile([16, 4 * H], F32, tag="g_sb")
                    gih_s = g_pre
                    for fc in range(4):
                        gslc = slice(fc * BLKW, (fc + 1) * BLKW)
                        func = AF.Tanh if fc == 2 else AF.Sigmoid
                        if s == 0:
                            nc.scalar.activation(g_sb[:, gslc], gih_s[:, gslc], func)
                            continue
                        ps = gps.tile([16, BLKW], F32, tag="g_ps")
                        nc.tensor.matmul(ps, idnr_sb, gih_s[:, gslc],
                                         start=True, stop=False)
                        for k in range(DC):
                            nc.tensor.matmul(
                                ps, r32(outsT[:, k, (s - 1) * 16:s * 16]),
                                r32(wlhht_sb[:, k, fc * BLKW:(fc + 1) * BLKW]),
                                start=False, stop=(k == DC - 1))
                        nc.scalar.activation(g_sb[:, gslc], ps, func)
                    i_g = g_sb[:, 0:H]
                    f_g = g_sb[:, H:2 * H]
                    g_g = g_sb[:, 2 * H:3 * H]
                    o_g = g_sb[:, 3 * H:4 * H]
                    c_new = lcp.tile([16, 4 * H], F32, tag="c")
                    nc.vector.tensor_mul(c_new[:, 0:H], f_g, c_prev[:, 0:H])
                    nc.vector.tensor_mul(c_new[:, H:2 * H], i_g, g_g)
                    nc.vector.tensor_add(c_new[:, 0:H], c_new[:, 0:H], c_new[:, H:2 * H])
                    h_sb = lt.tile([16, H], F32, tag="h_sb")
                    nc.scalar.activation(h_sb, c_new[:, 0:H], AF.Tanh)
                    nc.vector.tensor_mul(h_sb, o_g, h_sb)
                    c_prev = c_new
                    for k in range(DC):
                        tp = tps.tile([128, 16], F32, tag="tp")
                        nc.tensor.transpose(tp, h_sb[:, k * 128:(k + 1) * 128], idn_sb)
                        nc.vector.tensor_copy(outsT[:, k, s * 16:(s + 1) * 16], tp)

                lab_ps = lps.tile([C, SN], F32, tag="lab_ps")
                for k in range(DC):
                    nc.tensor.matmul(lab_ps, r32(wrt_sb[:, k, :]), r32(outsT[:, k, :]),
                                     start=(k == 0), stop=(k == DC - 1))
                lab_sb = lt.tile([C, SN], F32, tag="h_sb")
                nc.scalar.activation(lab_sb, lab_ps, AF.Identity, bias=brc_sb)
                nc.sync.dma_start(out=lab_out, in_=lab_sb)

    nc.compile()
    return nc


_NC = None


def get_nc():
    global _NC
    if _NC is None:
        _NC = build_kernel()
    return _NC


def prep_core_inputs(b, node_resnet, edge_resnet, node_num_rec, W1, b1, W2, b2,
                     Wm, bm, Wih, Whh, bih, bhh, Wl_ih, Wl_hh, bl_ih, bl_hh, Wr, br):
    f4 = np.float32
    nn_ = np.asarray(node_num_rec[b])                       # [S]
    mask = (np.arange(N)[None, :] < nn_[:, None])           # [S,N] bool
    emask = mask[:, :, None] & mask[:, None, :]             # [S,N,N]
    offdiag = ~np.eye(N, dtype=bool)

    node = np.asarray(node_resnet[b], f4) * mask[:, None, :]          # [S,D,N]
    edge = np.asarray(edge_resnet[b], f4) * (emask & offdiag)[:, None, :, :]

    edge_t = np.ascontiguousarray(edge.transpose(1, 0, 2, 3)).reshape(D, SNN)
    node_t = np.ascontiguousarray(node.transpose(1, 0, 2)).reshape(D, SN)
    emf = emask.astype(f4).reshape(1, SNN)
    negmb2 = (np.float32(b2[0]) + (1.0 - emf) * np.float32(NEG)).astype(f4)

    consts = np.zeros((128, 536), f4)
    consts[:, 0:512] = np.broadcast_to(mask.astype(f4).reshape(1, SN), (128, SN))
    consts[:, 512:516] = np.asarray(b1, f4).reshape(DC, 128).T
    consts[:, 516:520] = np.asarray(bm, f4).reshape(DC, 128).T
    consts[:, 520:528] = (np.asarray(bih, f4) + np.asarray(bhh, f4))[:1024].reshape(8, 128).T
    consts[:, 528:532] = np.asarray(bih, f4)[1024:].reshape(DC, 128).T
    consts[:, 532:536] = np.asarray(bhh, f4)[1024:].reshape(DC, 128).T

    ins = {
        "edge": edge_t, "node0": node_t, "negmb2": negmb2,
        "w1t": np.ascontiguousarray(np.asarray(W1, f4).T),
        "w2rep": np.ascontiguousarray(np.repeat(np.asarray(W2, f4).T, 128, axis=1)),
        "wmet": np.ascontiguousarray(np.asarray(Wm[:, D:], f4).T),
        "wmht": np.ascontiguousarray(np.asarray(Wm[:, :D], f4).T),
        "wiht": np.ascontiguousarray(np.asarray(Wih, f4).T),
        "whht": np.ascontiguousarray(np.asarray(Whh, f4).T),
        "consts": consts,
        "ones1": np.ones((1, 128), f4),
        "wliht": np.ascontiguousarray(np.asarray(Wl_ih, f4).T),
        "wlhht": np.ascontiguousarray(np.asarray(Wl_hh, f4).T),
        "blrow": (np.asarray(bl_ih, f4) + np.asarray(bl_hh, f4)).reshape(1, 4 * H),
        "wrt": np.ascontiguousarray(np.asarray(Wr, f4).T),
        "brc": np.asarray(br, f4).reshape(C, 1),
        "idn16": np.eye(16, dtype=f4),
        "idn16r": np.eye(16, dtype=f4),
    }
    post = {"emask": emf.reshape(S, N, N), "nmask": mask}
    return ins, post


def kernel(node_resnet, edge_resnet, node_num_rec, W1, b1, W2, b2, Wm, bm,
           Wih, Whh, bih, bhh, Wl_ih, Wl_hh, bl_ih, bl_hh, Wr, br,
           _trace=False):
    nc = get_nc()
    args = (node_resnet, edge_resnet, node_num_rec, W1, b1, W2, b2, Wm, bm,
            Wih, Whh, bih, bhh, Wl_ih, Wl_hh, bl_ih, bl_hh, Wr, br)
    in_maps, posts = [], []
    for b in range(B):
        ins, post = prep_core_inputs(b, *args)
        in_maps.append(ins)
        posts.append(post)

    res = run_bass_kernel_spmd(nc, in_maps, core_ids=list(range(B)), trace=_trace)

    adj = np.zeros((B, S, N, N), np.float32)
    label = np.zeros((B, S, N, C), np.float32)
    for b in range(B):
        out = res.results[b]
        em = posts[b]["emask"]
        nm = posts[b]["nmask"]
        gate = out["adj_out"].reshape(S, N, N)
        adj[b] = gate + 0.5 * (1.0 - em)
        lab = out["lab_out"].reshape(C, S, N).transpose(1, 2, 0)
        label[b] = lab * nm[:, :, None]
    if _trace:
        kernel.last_exec_time_ns = res.exec_time_ns
        kernel.last_results = res
    return adj, label
